# revision 1
# baseline (speedup 1.0000x reference)
"""GATv2 2-layer encoder on 8 TRN2 NeuronCores — v2.

Destination-node sharding in a permuted "slot" space: nodes are bin-packed
into 392 tiles of 128 slots (balancing in-edge counts), 49 tiles per core.
All full node tables (x, xl1, xl2) use a chunk-major row layout so the xl2
AllGather can be split into chunks and overlapped with the layer-1 edge
loop. Per tile, all bpt*128 edge rows are fetched with single batched
indirect DMAs (multi-column offset APs); e-embedding rows are summed onto
the xl rows in the DMA datapath (compute_op=add). One-hot scatter/broadcast
matrices are generated on-chip (iota + is_equal + PE transpose). Layer 1
needs no collective: each core computes the full xl1 table locally.
"""
import sys
import heapq

import numpy as np

sys.path.insert(0, "/opt/trn_rl_repo")

import ml_dtypes  # noqa: E402
import concourse.bass as bass  # noqa: E402
import concourse.tile as tile  # noqa: E402
from concourse import bacc, mybir  # noqa: E402
from concourse.bass_utils import run_bass_kernel_spmd  # noqa: E402
from concourse.masks import make_identity  # noqa: E402

N, E, R = 50000, 400000, 500
IN, HID, H, OUT = 128, 64, 4, 128
HC1, HC2 = H * HID, H * OUT  # 256, 512
W = 8            # cores
P = 128          # partitions / tile slots / edge-block size
NT = 49          # node tiles per core
TILES = W * NT   # 392
NSLOT = TILES * P  # 50176
SHARD = NT * P   # 6272 rows per core
RPAD = 512       # padded relation table rows (rows R.. are zero)

F32 = mybir.dt.float32
BF16 = mybir.dt.bfloat16
I32 = mybir.dt.int32
BF = ml_dtypes.bfloat16

# xl2 AllGather chunking (tiles per chunk); sum must be NT
CHUNK_TILES = [49]
CHUNK_T0 = np.cumsum([0] + CHUNK_TILES[:-1]).tolist()
CHUNK_BASE = np.cumsum(
    [0] + [W * ct * P for ct in CHUNK_TILES[:-1]]).tolist()
NCHUNK = len(CHUNK_TILES)


def _chunk_of_tile(t):
    for k in range(NCHUNK):
        if CHUNK_T0[k] <= t < CHUNK_T0[k] + CHUNK_TILES[k]:
            return k
    raise AssertionError


def _row0_of_tile(g):
    """First chunk-major DRAM row of global tile g in a full node table."""
    c, t = g // NT, g % NT
    k = _chunk_of_tile(t)
    return CHUNK_BASE[k] + c * CHUNK_TILES[k] * P + (t - CHUNK_T0[k]) * P


_LUT = None


def _chunkrow_lut():
    """slot (tile-major) -> chunk-major DRAM row."""
    global _LUT
    if _LUT is None:
        lut = np.empty(NSLOT, np.int64)
        for g in range(TILES):
            lut[g * P:(g + 1) * P] = _row0_of_tile(g) + np.arange(P)
        _LUT = lut
    return _LUT


def _preprocess(edge_index):
    """Self-loops, balanced node->tile binning, per-core index planes."""
    src = np.asarray(edge_index[0], dtype=np.int64)
    rel = np.asarray(edge_index[1], dtype=np.int64)
    dst = np.asarray(edge_index[2], dtype=np.int64)
    loop = np.arange(N, dtype=np.int64)
    src_f = np.concatenate([src, loop])
    dst_f = np.concatenate([dst, loop])
    rel_f = np.concatenate([rel, np.full(N, R, dtype=np.int64)])

    deg = np.bincount(dst_f, minlength=N)

    order = np.argsort(-deg, kind="stable")
    tile_of = np.empty(N, np.int64)
    slot_of = np.empty(N, np.int64)
    heap = [(0, t) for t in range(TILES)]
    heapq.heapify(heap)
    counts = np.zeros(TILES, np.int64)
    loads = np.zeros(TILES, np.int64)
    for n in order:
        while True:
            load, t = heapq.heappop(heap)
            if counts[t] < P:
                break
        tile_of[n] = t
        slot_of[n] = counts[t]
        counts[t] += 1
        loads[t] += deg[n]
        if counts[t] < P:
            heapq.heappush(heap, (loads[t], t))

    perm_pos = tile_of * P + slot_of          # node -> slot (tile-major)
    lut = _chunkrow_lut()

    bpt = max(1, int(-(-loads.max() // P)))   # blocks per tile (uniform)
    nblk = NT * bpt
    cap = bpt * P

    et = tile_of[dst_f]
    eorder = np.argsort(et, kind="stable")
    et_s = et[eorder]
    starts = np.searchsorted(et_s, np.arange(TILES))
    ends = np.searchsorted(et_s, np.arange(TILES), side="right")

    src_a = np.zeros((TILES, cap), np.int64)          # pad -> row 0
    rel_a = np.full((TILES, cap), R, np.int64)        # pad -> zero e-row
    seg_a = np.full((TILES, cap), 999, np.int64)      # pad -> no one-hot
    for t in range(TILES):
        idx = eorder[starts[t]:ends[t]]
        k = idx.shape[0]
        src_a[t, :k] = lut[perm_pos[src_f[idx]]]
        rel_a[t, :k] = rel_f[idx]
        seg_a[t, :k] = slot_of[dst_f[idx]]

    gsrc = np.zeros((W, P, nblk), np.int32)
    grel = np.zeros((W, P, nblk), np.int32)
    gseg = np.zeros((W, P, nblk), np.float32)
    for c in range(W):
        g0 = c * NT
        gsrc[c] = src_a[g0:g0 + NT].reshape(nblk, P).T
        grel[c] = rel_a[g0:g0 + NT].reshape(nblk, P).T
        gseg[c] = seg_a[g0:g0 + NT].reshape(nblk, P).T.astype(np.float32)

    return dict(bpt=bpt, nblk=nblk, perm_pos=perm_pos, lut=lut,
                gsrc=gsrc, grel=grel, gseg=gseg)


def _build(bpt, reps=1):
    nblk = NT * bpt
    nc = bacc.Bacc("TRN2", target_bir_lowering=False, debug=False, num_devices=W)

    # ---- per-core inputs
    gsrc = nc.declare_dram_parameter("gsrc", [P, nblk], I32, isOutput=False)
    gseg = nc.declare_dram_parameter("gseg", [P, nblk], F32, isOutput=False)
    gsegT = nc.declare_dram_parameter("gsegT", [nblk, P], BF16, isOutput=False)
    relE = nc.declare_dram_parameter("relE", [nblk * P, IN], BF16, isOutput=False)
    x_own = nc.declare_dram_parameter("x_own", [SHARD, IN], BF16, isOutput=False)
    # ---- replicated inputs
    wl1 = nc.declare_dram_parameter("wl1", [IN, HC1], F32, isOutput=False)
    wr1 = nc.declare_dram_parameter("wr1", [IN, HC1], F32, isOutput=False)
    we1 = nc.declare_dram_parameter("we1", [IN, HC1], F32, isOutput=False)
    att1f = nc.declare_dram_parameter("att1f", [1, HC1], F32, isOutput=False)
    eb1 = nc.declare_dram_parameter("eb1", [1, HC1], F32, isOutput=False)
    ob1 = nc.declare_dram_parameter("ob1", [1, HC1], F32, isOutput=False)
    wl2 = nc.declare_dram_parameter("wl2", [HC1, HC2], F32, isOutput=False)
    wr2 = nc.declare_dram_parameter("wr2", [HC1, HC2], F32, isOutput=False)
    we2 = nc.declare_dram_parameter("we2", [IN, HC2], F32, isOutput=False)
    att2f = nc.declare_dram_parameter("att2f", [1, HC2], F32, isOutput=False)
    eb2 = nc.declare_dram_parameter("eb2", [1, HC2], F32, isOutput=False)
    ob2 = nc.declare_dram_parameter("ob2", [1, OUT], F32, isOutput=False)
    out_p = nc.declare_dram_parameter("out", [SHARD, OUT], F32, isOutput=True)
    import os as _os
    _DBG = _os.environ.get("GAT_DEBUG", "0") == "1"
    if _DBG:
        dbgV = nc.declare_dram_parameter("dbgV", [P, HC1], BF16, isOutput=True)
        dbgMf = nc.declare_dram_parameter("dbgMf", [P, HC1], BF16, isOutput=True)
        dbglg = nc.declare_dram_parameter("dbglg", [P, 3 * H], F32, isOutput=True)
        dbgwf = nc.declare_dram_parameter("dbgwf", [P, HC1], BF16, isOutput=True)
        dbgRv = nc.declare_dram_parameter("dbgRv", [P, HC1], BF16, isOutput=True)
        dbgQ = nc.declare_dram_parameter("dbgQ", [P, P], BF16, isOutput=True)
        dbgh = nc.declare_dram_parameter("dbgh", [P, HC1], BF16, isOutput=True)
        dbgxr = nc.declare_dram_parameter("dbgxr", [P, HC1], BF16, isOutput=True)

    # ---- internal DRAM
    xl1_shard = nc.dram_tensor("xl1_shard", [SHARD, HC1], BF16)
    xl1_full = nc.dram_tensor("xl1_full", [NSLOT, HC1], BF16,
                              addr_space="Shared")
    h_shard = nc.dram_tensor("h_shard", [SHARD, HC1], BF16)
    xl2_shard = nc.dram_tensor("xl2_shard", [SHARD, HC2], BF16)
    xl2_full = nc.dram_tensor("xl2_full", [NSLOT, HC2], BF16, addr_space="Shared")

    RG = [list(range(W))]
    IOA = bass.IndirectOffsetOnAxis
    ACTF = mybir.ActivationFunctionType
    ALU = mybir.AluOpType
    K1 = 3 if bpt % 3 == 0 else 1       # superblock width, layer 1
    K2 = 3 if bpt % 3 == 0 else 2       # superblock width, layer 2

    def sb_splits(K):
        sp, j = [], 0
        while j < bpt:
            kk = min(K, bpt - j)
            sp.append((j, kk))
            j += kk
        return sp

    with tile.TileContext(nc) as tc:
        with tc.tile_pool(name="const", bufs=1) as cp:
            iotaF = cp.tile([P, P], F32, tag="iotaF")
            nc.gpsimd.iota(iotaF[:], pattern=[[1, P]], base=0,
                           channel_multiplier=0,
                           allow_small_or_imprecise_dtypes=True)
            iotaP = cp.tile([P, 1], F32, tag="iotaP")
            nc.gpsimd.iota(iotaP[:], pattern=[[1, 1]], base=0,
                           channel_multiplier=1,
                           allow_small_or_imprecise_dtypes=True)
            wl1b = cp.tile([IN, HC1], BF16, tag="wl1b")
            nc.gpsimd.dma_start(out=wl1b[:], in_=wl1[:])
            wr1b = cp.tile([IN, HC1], BF16, tag="wr1b")
            nc.gpsimd.dma_start(out=wr1b[:], in_=wr1[:])
            we1b = cp.tile([IN, HC1], BF16, tag="we1b")
            nc.gpsimd.dma_start(out=we1b[:], in_=we1[:])
            we2b = cp.tile([IN, HC2], BF16, tag="we2b")
            nc.gpsimd.dma_start(out=we2b[:], in_=we2[:])
            wl2b, wr2b = [], []
            for k in range(2):
                wl2bk = cp.tile([P, HC2], BF16, tag=f"wl2b{k}")
                nc.gpsimd.dma_start(out=wl2bk[:], in_=wl2[k * P:(k + 1) * P, :])
                wl2b.append(wl2bk)
                wr2bk = cp.tile([P, HC2], BF16, tag=f"wr2b{k}")
                nc.gpsimd.dma_start(out=wr2bk[:], in_=wr2[k * P:(k + 1) * P, :])
                wr2b.append(wr2bk)
            attB1 = cp.tile([P, K1 * HC1], BF16, tag="attB1")
            for j in range(K1):
                nc.gpsimd.dma_start(out=attB1[:, j * HC1:(j + 1) * HC1],
                                    in_=att1f[:].to_broadcast([P, HC1]))
            attB2 = cp.tile([P, K2 * HC2], BF16, tag="attB2")
            for j in range(K2):
                nc.gpsimd.dma_start(out=attB2[:, j * HC2:(j + 1) * HC2],
                                    in_=att2f[:].to_broadcast([P, HC2]))
            eb1B = cp.tile([P, HC1], F32, tag="eb1B")
            nc.sync.dma_start(out=eb1B[:], in_=eb1[:].to_broadcast([P, HC1]))
            ob1B = cp.tile([P, HC1], BF16, tag="ob1B")
            nc.gpsimd.dma_start(out=ob1B[:], in_=ob1[:].to_broadcast([P, HC1]))
            eb2B = cp.tile([P, HC2], F32, tag="eb2B")
            nc.sync.dma_start(out=eb2B[:], in_=eb2[:].to_broadcast([P, HC2]))
            ob2B = cp.tile([P, OUT], F32, tag="ob2B")
            nc.sync.dma_start(out=ob2B[:], in_=ob2[:].to_broadcast([P, OUT]))
            gsrc_t = cp.tile([P, nblk], I32, tag="gsrc_t")
            nc.sync.dma_start(out=gsrc_t[:], in_=gsrc[:])
            gseg_t = cp.tile([P, nblk], F32, tag="gseg_t")
            nc.sync.dma_start(out=gseg_t[:], in_=gseg[:])
            xr1_loc = cp.tile([P, NT * HC1], BF16, tag="xr1_loc")
            xr2_loc = cp.tile([P, NT * HC2], BF16, tag="xr2_loc")

            for _rep in range(reps):
                # ======== phase A: e-tables + xr1 + xl1_full ========
                with (
                    tc.tile_pool(name="pa_w", bufs=3) as wp,
                    tc.tile_pool(name="pa_p", bufs=2, space="PSUM") as pp,
                ):
                    # own shard: xr1 (kept in SBUF) + xl1 (sharded, then
                    # AllGathered). Chunk layout is single-chunk so the
                    # AllGather concat order matches the chunk-major rows.
                    for t in range(NT):
                        xoT = wp.tile([P, P], BF16, tag="xoT")
                        nc.sync.dma_start(out=xoT[:],
                                          in_=x_own[t * P:(t + 1) * P, :],
                                          transpose=True)
                        psR = pp.tile([P, HC1], F32, tag="psR")
                        nc.tensor.matmul(psR[:], lhsT=xoT[:], rhs=wr1b[:],
                                         start=True, stop=True)
                        nc.vector.tensor_tensor(
                            out=xr1_loc[:, t * HC1:(t + 1) * HC1],
                            in0=psR[:], in1=eb1B[:], op=ALU.add)
                        psL = pp.tile([P, HC1], F32, tag="psL")
                        nc.tensor.matmul(psL[:], lhsT=xoT[:], rhs=wl1b[:],
                                         start=True, stop=True)
                        xls = wp.tile([P, HC1], BF16, tag="xls")
                        if t % 2 == 0:
                            nc.scalar.activation(xls[:], psL[:], ACTF.Copy)
                        else:
                            nc.vector.tensor_copy(xls[:], psL[:])
                        nc.sync.dma_start(
                            out=xl1_shard[t * P:(t + 1) * P, :], in_=xls[:])
                    nc.gpsimd.collective_compute(
                        "AllGather", ALU.bypass,
                        ins=[xl1_shard[:]], outs=[xl1_full[:]],
                        replica_groups=RG)

                # ======== layer 1 edge loop (+ fused layer-2 transforms) ====
                with (
                    tc.tile_pool(name="l1_w", bufs=3) as wp,
                    tc.tile_pool(name="l1_p", bufs=2, space="PSUM") as pp,
                    tc.tile_pool(name="l1_a", bufs=2, space="PSUM") as pa,
                ):
                    for t in range(NT):
                        b0 = t * bpt
                        relT = wp.tile([P, bpt * P], BF16, tag="relT")
                        nc.sync.dma_start(
                            out=relT[:],
                            in_=relE[b0 * P:(b0 + bpt) * P, :],
                            transpose=True)

                        acc1t = pa.tile([P, HC1], F32, tag="acc1t", bufs=2)
                        accd1t = pa.tile([P, H], F32, tag="accd1t", bufs=2)
                        acc1 = acc1t[:]
                        accd1 = accd1t[:]
                        XR1 = xr1_loc[:, t * HC1:(t + 1) * HC1]

                        for (j0, kk) in sb_splits(K1):
                            sfx = "" if kk == K1 else f"_{kk}"
                            V1 = wp.tile([P, kk * HC1], BF16, tag=f"V1{sfx}")
                            for jj in range(kk):
                                b = b0 + j0 + jj
                                nc.gpsimd.indirect_dma_start(
                                    out=V1[:, jj * HC1:(jj + 1) * HC1],
                                    out_offset=None, in_=xl1_full[:],
                                    in_offset=IOA(ap=gsrc_t[:, b:b + 1],
                                                  axis=0))
                            Qt = wp.tile([P, kk * P], BF16, tag=f"Qt{sfx}")
                            for jj in range(kk):
                                nc.vector.tensor_scalar(
                                    out=Qt[:, jj * P:(jj + 1) * P],
                                    in0=iotaF[:],
                                    scalar1=gseg_t[:, b0 + j0 + jj:b0 + j0 + jj + 1],
                                    scalar2=None, op0=ALU.is_equal)
                            segB = wp.tile([P, kk * P], BF16, tag=f"segB{sfx}")
                            nc.scalar.dma_start(
                                out=segB[:],
                                in_=gsegT[b0 + j0:b0 + j0 + kk, :]
                                .rearrange("(o k) p -> o (k p)", o=1)
                                .to_broadcast([P, kk * P]))
                            Pbt = wp.tile([P, kk * P], BF16, tag=f"Pbt{sfx}")
                            nc.vector.tensor_scalar(
                                out=Pbt[:], in0=segB[:], scalar1=iotaP[:],
                                scalar2=None, op0=ALU.is_equal)
                            psM = pp.tile([P, kk * HC1], F32, tag=f"psM{sfx}", bufs=2)
                            for jj in range(kk):
                                j = j0 + jj
                                nc.tensor.matmul(
                                    psM[:, jj * HC1:(jj + 1) * HC1],
                                    lhsT=Pbt[:, jj * P:(jj + 1) * P],
                                    rhs=XR1, start=True, stop=False)
                                nc.tensor.matmul(
                                    psM[:, jj * HC1:(jj + 1) * HC1],
                                    lhsT=relT[:, j * P:(j + 1) * P],
                                    rhs=we1b[:], start=False, stop=True)
                            Mf = wp.tile([P, kk * HC1], BF16, tag=f"Mf{sfx}")
                            nc.vector.scalar_tensor_tensor(
                                out=Mf[:], in0=psM[:], scalar=1.0,
                                in1=V1[:], op0=ALU.mult, op1=ALU.add)
                            Mr = wp.tile([P, kk * HC1], BF16, tag=f"Mr{sfx}")
                            nc.scalar.activation(Mr[:], Mf[:], ACTF.Prelu,
                                                 alpha=0.2)
                            Tm = wp.tile([P, kk * HC1], BF16, tag=f"Tm{sfx}")
                            nc.vector.tensor_tensor(
                                out=Tm[:], in0=Mr[:],
                                in1=attB1[:, 0:kk * HC1], op=ALU.mult)
                            logit = wp.tile([P, kk * H], F32, tag=f"lg{sfx}")
                            nc.vector.tensor_reduce(
                                out=logit[:],
                                in_=Tm[:].rearrange("p (q c) -> p q c", c=HID),
                                axis=mybir.AxisListType.X, op=ALU.add)
                            wfb = wp.tile([P, kk * HC1], BF16, tag=f"wfb{sfx}")
                            nc.scalar.activation(
                                wfb[:].rearrange("p (q c) -> p q c", c=HID),
                                logit[:].rearrange("p (q o) -> p q o", o=1)
                                .to_broadcast([P, kk * H, HID]),
                                ACTF.Exp)
                            Rv = wp.tile([P, kk * HC1], BF16, tag=f"Rv{sfx}")
                            nc.gpsimd.tensor_tensor(
                                out=Rv[:], in0=wfb[:], in1=V1[:],
                                op=ALU.mult)
                            Wc = wp.tile([P, kk * H], BF16, tag=f"Wc{sfx}")
                            nc.vector.tensor_copy(
                                Wc[:],
                                wfb[:].rearrange("p (q c) -> p q c", c=HID)
                                [:, :, 0])
                            if _DBG and t == 0 and j0 == 0:
                                nc.sync.dma_start(out=dbgV[:], in_=V1[:, 0:HC1])
                                nc.sync.dma_start(out=dbgMf[:], in_=Mf[:, 0:HC1])
                                nc.sync.dma_start(out=dbglg[:], in_=logit[:, 0:3 * H])
                                nc.sync.dma_start(out=dbgwf[:], in_=wfb[:, 0:HC1])
                                nc.sync.dma_start(out=dbgRv[:], in_=Rv[:, 0:HC1])
                                nc.sync.dma_start(out=dbgQ[:], in_=Qt[:, 0:P])
                            for jj in range(kk):
                                j = j0 + jj
                                nc.tensor.matmul(
                                    acc1, lhsT=Qt[:, jj * P:(jj + 1) * P],
                                    rhs=Rv[:, jj * HC1:(jj + 1) * HC1],
                                    start=(j == 0), stop=(j == bpt - 1))
                                nc.tensor.matmul(
                                    accd1, lhsT=Qt[:, jj * P:(jj + 1) * P],
                                    rhs=Wc[:, jj * H:(jj + 1) * H],
                                    start=(j == 0), stop=(j == bpt - 1))

                        # epilogue: h, then xl2/xr2 transforms for this tile
                        dn1 = wp.tile([P, H], F32, tag="dn1")
                        nc.vector.tensor_scalar_add(dn1[:], accd1, 1e-20)
                        rec = wp.tile([P, H], F32, tag="rec")
                        nc.vector.reciprocal(rec[:], dn1[:])
                        htmp = wp.tile([P, HC1], BF16, tag="htmp")
                        for hh in range(H):
                            nc.scalar.activation(
                                htmp[:, hh * HID:(hh + 1) * HID],
                                acc1[:, hh * HID:(hh + 1) * HID],
                                ACTF.Copy, scale=rec[:, hh:hh + 1])
                        hsb = wp.tile([P, HC1], BF16, tag="hsb")
                        nc.vector.tensor_tensor(out=hsb[:], in0=htmp[:],
                                                in1=ob1B[:], op=ALU.add)
                        if _DBG and t == 0:
                            nc.sync.dma_start(out=dbgh[:], in_=hsb[:])
                            nc.sync.dma_start(
                                out=dbgxr[:], in_=xr1_loc[:, 0:HC1])
                        nc.sync.dma_start(out=h_shard[t * P:(t + 1) * P, :],
                                          in_=hsb[:])
                        hTt = wp.tile([P, HC1], BF16, tag="hTt")
                        for k in range(2):
                            nc.sync.dma_start(
                                out=hTt[:, k * P:(k + 1) * P],
                                in_=h_shard[t * P:(t + 1) * P,
                                            k * P:(k + 1) * P],
                                transpose=True)
                        psC2 = pp.tile([P, K1 * HC1], F32, tag="psM", bufs=2)
                        for k in range(2):
                            nc.tensor.matmul(psC2[:, 0:HC2],
                                             lhsT=hTt[:, k * P:(k + 1) * P],
                                             rhs=wl2b[k][:],
                                             start=(k == 0), stop=(k == 1))
                        xl2sb = wp.tile([P, HC2], BF16, tag="xl2sb")
                        nc.scalar.activation(xl2sb[:], psC2[:, 0:HC2], ACTF.Copy)
                        nc.sync.dma_start(out=xl2_shard[t * P:(t + 1) * P, :],
                                          in_=xl2sb[:])
                        psC3 = pp.tile([P, K1 * HC1], F32, tag="psM", bufs=2)
                        for k in range(2):
                            nc.tensor.matmul(psC3[:, 0:HC2],
                                             lhsT=hTt[:, k * P:(k + 1) * P],
                                             rhs=wr2b[k][:],
                                             start=(k == 0), stop=(k == 1))
                        nc.vector.tensor_tensor(
                            out=xr2_loc[:, t * HC2:(t + 1) * HC2],
                            in0=psC3[:, 0:HC2], in1=eb2B[:], op=ALU.add)

                        # chunked AllGather as soon as a chunk's tiles are done
                        for k in range(NCHUNK):
                            if t == CHUNK_T0[k] + CHUNK_TILES[k] - 1:
                                r0 = CHUNK_T0[k] * P
                                r1 = r0 + CHUNK_TILES[k] * P
                                o0 = CHUNK_BASE[k]
                                o1 = o0 + W * CHUNK_TILES[k] * P
                                nc.gpsimd.collective_compute(
                                    "AllGather", ALU.bypass,
                                    ins=[xl2_shard[r0:r1, :]],
                                    outs=[xl2_full[o0:o1, :]],
                                    replica_groups=RG)

                # ======== layer 2 edge loop ========
                with (
                    tc.tile_pool(name="l2_w", bufs=3) as wp,
                    tc.tile_pool(name="l2_p", bufs=2, space="PSUM") as pp,
                    tc.tile_pool(name="l2_a", bufs=1, space="PSUM") as pa,
                ):
                    for t in range(NT):
                        b0 = t * bpt
                        relT = wp.tile([P, bpt * P], BF16, tag="relT2")
                        nc.sync.dma_start(
                            out=relT[:],
                            in_=relE[b0 * P:(b0 + bpt) * P, :],
                            transpose=True)

                        acc2 = pa.tile([P, HC2], F32, tag="acc2", bufs=2)
                        accd2 = pa.tile([P, H], F32, tag="accd2", bufs=2)
                        XR2 = xr2_loc[:, t * HC2:(t + 1) * HC2]

                        for (j0, kk) in sb_splits(K2):
                            sfx = "" if kk == K2 else f"_{kk}"
                            V2 = wp.tile([P, kk * HC2], BF16, tag=f"V2{sfx}")
                            for jj in range(kk):
                                b = b0 + j0 + jj
                                nc.gpsimd.indirect_dma_start(
                                    out=V2[:, jj * HC2:(jj + 1) * HC2],
                                    out_offset=None, in_=xl2_full[:],
                                    in_offset=IOA(ap=gsrc_t[:, b:b + 1],
                                                  axis=0))
                            Qt = wp.tile([P, kk * P], BF16, tag=f"Qt{sfx}")
                            for jj in range(kk):
                                nc.vector.tensor_scalar(
                                    out=Qt[:, jj * P:(jj + 1) * P],
                                    in0=iotaF[:],
                                    scalar1=gseg_t[:, b0 + j0 + jj:b0 + j0 + jj + 1],
                                    scalar2=None, op0=ALU.is_equal)
                            segB = wp.tile([P, kk * P], BF16, tag=f"segB{sfx}")
                            nc.scalar.dma_start(
                                out=segB[:],
                                in_=gsegT[b0 + j0:b0 + j0 + kk, :]
                                .rearrange("(o k) p -> o (k p)", o=1)
                                .to_broadcast([P, kk * P]))
                            Pbt = wp.tile([P, kk * P], BF16, tag=f"Pbt{sfx}")
                            nc.vector.tensor_scalar(
                                out=Pbt[:], in0=segB[:], scalar1=iotaP[:],
                                scalar2=None, op0=ALU.is_equal)
                            psM = pp.tile([P, kk * HC2], F32, tag=f"psM{sfx}", bufs=1)
                            for jj in range(kk):
                                j = j0 + jj
                                nc.tensor.matmul(
                                    psM[:, jj * HC2:(jj + 1) * HC2],
                                    lhsT=Pbt[:, jj * P:(jj + 1) * P],
                                    rhs=XR2, start=True, stop=False)
                                nc.tensor.matmul(
                                    psM[:, jj * HC2:(jj + 1) * HC2],
                                    lhsT=relT[:, j * P:(j + 1) * P],
                                    rhs=we2b[:], start=False, stop=True)
                            Mf = wp.tile([P, kk * HC2], BF16, tag=f"Mf{sfx}")
                            nc.vector.scalar_tensor_tensor(
                                out=Mf[:], in0=psM[:], scalar=1.0,
                                in1=V2[:], op0=ALU.mult, op1=ALU.add)
                            Mr = wp.tile([P, kk * HC2], BF16, tag=f"Mr{sfx}")
                            nc.scalar.activation(Mr[:], Mf[:], ACTF.Prelu,
                                                 alpha=0.2)
                            Tm = wp.tile([P, kk * HC2], BF16, tag=f"Tm{sfx}")
                            nc.vector.tensor_tensor(
                                out=Tm[:], in0=Mr[:],
                                in1=attB2[:, 0:kk * HC2], op=ALU.mult)
                            logit = wp.tile([P, kk * H], F32, tag=f"lg{sfx}")
                            nc.vector.tensor_reduce(
                                out=logit[:],
                                in_=Tm[:].rearrange("p (q c) -> p q c", c=OUT),
                                axis=mybir.AxisListType.X, op=ALU.add)
                            wfb = wp.tile([P, kk * HC2], BF16, tag=f"wfb{sfx}")
                            nc.scalar.activation(
                                wfb[:].rearrange("p (q c) -> p q c", c=OUT),
                                logit[:].rearrange("p (q o) -> p q o", o=1)
                                .to_broadcast([P, kk * H, OUT]),
                                ACTF.Exp)
                            Rv = wp.tile([P, kk * HC2], BF16, tag=f"Rv{sfx}")
                            nc.gpsimd.tensor_tensor(
                                out=Rv[:], in0=wfb[:], in1=V2[:],
                                op=ALU.mult)
                            Wc = wp.tile([P, kk * H], BF16, tag=f"Wc{sfx}")
                            nc.vector.tensor_copy(
                                Wc[:],
                                wfb[:].rearrange("p (q c) -> p q c", c=OUT)
                                [:, :, 0])
                            for jj in range(kk):
                                j = j0 + jj
                                nc.tensor.matmul(
                                    acc2[:], lhsT=Qt[:, jj * P:(jj + 1) * P],
                                    rhs=Rv[:, jj * HC2:(jj + 1) * HC2],
                                    start=(j == 0), stop=(j == bpt - 1))
                                nc.tensor.matmul(
                                    accd2[:], lhsT=Qt[:, jj * P:(jj + 1) * P],
                                    rhs=Wc[:, jj * H:(jj + 1) * H],
                                    start=(j == 0), stop=(j == bpt - 1))

                        # epilogue: out = mean_h(acc_h/denom_h) + bias
                        dn2 = wp.tile([P, H], F32, tag="dn2")
                        nc.vector.tensor_scalar_add(dn2[:], accd2[:], 1e-20)
                        rec2 = wp.tile([P, H], F32, tag="rec2")
                        nc.vector.reciprocal(rec2[:], dn2[:])
                        rec4 = wp.tile([P, H], F32, tag="rec4")
                        nc.vector.tensor_scalar_mul(rec4[:], rec2[:], 0.25)
                        ho = wp.tile([P, H * OUT], F32, tag="ho")
                        for hh in range(H):
                            nc.scalar.activation(
                                ho[:, hh * OUT:(hh + 1) * OUT],
                                acc2[:, hh * OUT:(hh + 1) * OUT],
                                ACTF.Copy, scale=rec4[:, hh:hh + 1])
                        s01 = wp.tile([P, OUT], F32, tag="s01")
                        nc.vector.tensor_tensor(out=s01[:], in0=ho[:, 0:OUT],
                                                in1=ho[:, OUT:2 * OUT],
                                                op=ALU.add)
                        s23 = wp.tile([P, OUT], F32, tag="s23")
                        nc.gpsimd.tensor_tensor(out=s23[:],
                                                in0=ho[:, 2 * OUT:3 * OUT],
                                                in1=ho[:, 3 * OUT:4 * OUT],
                                                op=ALU.add)
                        s03 = wp.tile([P, OUT], F32, tag="s03")
                        nc.vector.tensor_tensor(out=s03[:], in0=s01[:],
                                                in1=s23[:], op=ALU.add)
                        osb = wp.tile([P, OUT], F32, tag="osb")
                        nc.gpsimd.tensor_tensor(out=osb[:], in0=s03[:],
                                                in1=ob2B[:], op=ALU.add)
                        nc.sync.dma_start(out=out_p[t * P:(t + 1) * P, :],
                                          in_=osb[:])

    nc.compile()
    return nc


def _make_in_maps(inp, pre):
    f32 = np.float32
    lut = pre["lut"]
    perm_pos = pre["perm_pos"]
    x = np.asarray(inp["x"], f32)
    rel_padz = np.zeros((R + 1, IN), BF)
    rel_padz[:R] = np.asarray(inp["relations"], f32).astype(BF)
    a = lambda k: np.asarray(inp[k], f32)
    rep = dict(
        wl1=a("Wl1"), wr1=a("Wr1"), we1=a("We1"),
        att1f=a("att1").reshape(1, HC1),
        eb1=(a("bl1") + a("br1")).reshape(1, HC1),
        ob1=(a("bl1") + a("bias1")).reshape(1, HC1),
        wl2=a("Wl2"), wr2=a("Wr2"), we2=a("We2"),
        att2f=a("att2").reshape(1, HC2),
        eb2=(a("bl2") + a("br2")).reshape(1, HC2),
        ob2=(a("bl2").reshape(H, OUT).mean(axis=0) + a("bias2")).reshape(1, OUT),
    )
    # per-core own x rows (tile-major within the core)
    x_slot_tm = np.zeros((NSLOT, IN), BF)   # tile-major slot order
    x_slot_tm[perm_pos] = x.astype(BF)
    in_maps = []
    for c in range(W):
        m = dict(rep)
        m["gsrc"] = np.ascontiguousarray(pre["gsrc"][c])
        m["gseg"] = np.ascontiguousarray(pre["gseg"][c])
        m["gsegT"] = np.ascontiguousarray(pre["gseg"][c].T.astype(BF))
        m["relE"] = np.ascontiguousarray(
            rel_padz[np.minimum(pre["grel"][c].T.reshape(-1), R)])
        m["x_own"] = np.ascontiguousarray(
            x_slot_tm[c * SHARD:(c + 1) * SHARD])
        in_maps.append(m)
    return in_maps


_CACHE = {}


def kernel(x, edge_index, relations,
           Wl1, bl1, Wr1, br1, We1, att1, bias1,
           Wl2, bl2, Wr2, br2, We2, att2, bias2, **_unused):
    x = np.asarray(x, np.float32)
    edge_index = np.asarray(edge_index)
    relations = np.asarray(relations, np.float32)

    pre = _preprocess(edge_index)
    bpt = pre["bpt"]

    if bpt not in _CACHE:
        _CACHE[bpt] = _build(bpt)
    nc = _CACHE[bpt]

    in_maps = _make_in_maps(
        dict(x=x, relations=relations, Wl1=Wl1, bl1=bl1, Wr1=Wr1, br1=br1,
             We1=We1, att1=att1, bias1=bias1, Wl2=Wl2, bl2=bl2, Wr2=Wr2,
             br2=br2, We2=We2, att2=att2, bias2=bias2), pre)

    import os
    trace = os.environ.get("GAT_TRACE", "0") == "1"
    res = run_bass_kernel_spmd(nc, in_maps, list(range(W)), trace=trace)
    global LAST_EXEC_NS, LAST_RES
    LAST_EXEC_NS = res.exec_time_ns
    LAST_RES = res
    cat = np.concatenate([res.results[c]["out"] for c in range(W)], axis=0)
    return np.ascontiguousarray(cat[pre["perm_pos"]])


if __name__ == "__main__":
    pass



# revision 4
# speedup vs baseline: 29.3645x; 29.3645x over previous
"""GATv2 2-layer encoder on 8 TRN2 NeuronCores — v4.

Destination-node sharding in a permuted "slot" space: nodes are bin-packed
into 392 tiles of 128 slots (balancing in-edge counts), 49 tiles per core.
Layer 1 needs no collective: every core computes the full xl1 table locally
from a host-transposed x (one matmul per tile, no DMA transposes); only the
layer-2 source table is AllGathered.  Per edge block, xl[src] rows come via
single-column indirect DMAs; xr[dst] and the relation embedding are added
in PSUM via one-hot / transposed-relation matmuls, and the gathered xl rows
are merged with an identity matmul so the pre-activation message never
takes a DVE pass.  exp(logit) is computed at [E,H] (not broadcast to
[E,H*C]); the alpha*xl product is one DVE broadcast multiply; the softmax
denominator accumulates into spare columns of the same PSUM bank as the
numerator.  One-hot scatter/select matrices are built on-chip with single
broadcast is_equal ops per superblock.
"""
import sys
import heapq

import numpy as np

sys.path.insert(0, "/opt/trn_rl_repo")

import ml_dtypes  # noqa: E402
import concourse.bass as bass  # noqa: E402
import concourse.tile as tile  # noqa: E402
from concourse import bacc, mybir  # noqa: E402
from concourse.bass_utils import run_bass_kernel_spmd  # noqa: E402
from concourse.masks import make_identity  # noqa: E402

N, E, R = 50000, 400000, 500
IN, HID, H, OUT = 128, 64, 4, 128
HC1, HC2 = H * HID, H * OUT  # 256, 512
W = 8            # cores
P = 128          # partitions / tile slots / edge-block size
NT = 49          # node tiles per core
TILES = W * NT   # 392
NSLOT = TILES * P  # 50176
SHARD = NT * P   # 6272 rows per core
RPAD = 512       # padded relation table rows (rows R.. are zero)

F32 = mybir.dt.float32
BF16 = mybir.dt.bfloat16
I32 = mybir.dt.int32
BF = ml_dtypes.bfloat16

# xl2 AllGather chunking (tiles per chunk); sum must be NT
CHUNK_TILES = [49]
CHUNK_T0 = np.cumsum([0] + CHUNK_TILES[:-1]).tolist()
CHUNK_BASE = np.cumsum(
    [0] + [W * ct * P for ct in CHUNK_TILES[:-1]]).tolist()
NCHUNK = len(CHUNK_TILES)


def _chunk_of_tile(t):
    for k in range(NCHUNK):
        if CHUNK_T0[k] <= t < CHUNK_T0[k] + CHUNK_TILES[k]:
            return k
    raise AssertionError


def _row0_of_tile(g):
    """First chunk-major DRAM row of global tile g in a full node table."""
    c, t = g // NT, g % NT
    k = _chunk_of_tile(t)
    return CHUNK_BASE[k] + c * CHUNK_TILES[k] * P + (t - CHUNK_T0[k]) * P


_LUT = None


def _chunkrow_lut():
    """slot (tile-major) -> chunk-major DRAM row."""
    global _LUT
    if _LUT is None:
        lut = np.empty(NSLOT, np.int64)
        for g in range(TILES):
            lut[g * P:(g + 1) * P] = _row0_of_tile(g) + np.arange(P)
        _LUT = lut
    return _LUT


def _preprocess(edge_index):
    """Self-loops, balanced node->tile binning, per-core index planes."""
    src = np.asarray(edge_index[0], dtype=np.int64)
    rel = np.asarray(edge_index[1], dtype=np.int64)
    dst = np.asarray(edge_index[2], dtype=np.int64)
    loop = np.arange(N, dtype=np.int64)
    src_f = np.concatenate([src, loop])
    dst_f = np.concatenate([dst, loop])
    rel_f = np.concatenate([rel, np.full(N, R, dtype=np.int64)])

    deg = np.bincount(dst_f, minlength=N)

    order = np.argsort(-deg, kind="stable")
    tile_of = np.empty(N, np.int64)
    slot_of = np.empty(N, np.int64)
    heap = [(0, t) for t in range(TILES)]
    heapq.heapify(heap)
    counts = np.zeros(TILES, np.int64)
    loads = np.zeros(TILES, np.int64)
    for n in order:
        while True:
            load, t = heapq.heappop(heap)
            if counts[t] < P:
                break
        tile_of[n] = t
        slot_of[n] = counts[t]
        counts[t] += 1
        loads[t] += deg[n]
        if counts[t] < P:
            heapq.heappush(heap, (loads[t], t))

    perm_pos = tile_of * P + slot_of          # node -> slot (tile-major)
    lut = _chunkrow_lut()

    bpt = max(1, int(-(-loads.max() // P)))   # blocks per tile (uniform)
    nblk = NT * bpt
    cap = bpt * P

    et = tile_of[dst_f]
    eorder = np.argsort(et, kind="stable")
    et_s = et[eorder]
    starts = np.searchsorted(et_s, np.arange(TILES))
    ends = np.searchsorted(et_s, np.arange(TILES), side="right")

    src_a = np.zeros((TILES, cap), np.int64)          # pad -> row 0
    rel_a = np.full((TILES, cap), R, np.int64)        # pad -> zero e-row
    seg_a = np.full((TILES, cap), 999, np.int64)      # pad -> no one-hot
    for t in range(TILES):
        idx = eorder[starts[t]:ends[t]]
        k = idx.shape[0]
        src_a[t, :k] = lut[perm_pos[src_f[idx]]]
        rel_a[t, :k] = rel_f[idx]
        seg_a[t, :k] = slot_of[dst_f[idx]]

    gsrc = np.zeros((W, P, nblk), np.int32)
    grel = np.zeros((W, P, nblk), np.int32)
    gseg = np.zeros((W, P, nblk), np.float32)
    for c in range(W):
        g0 = c * NT
        gsrc[c] = src_a[g0:g0 + NT].reshape(nblk, P).T
        grel[c] = rel_a[g0:g0 + NT].reshape(nblk, P).T
        gseg[c] = seg_a[g0:g0 + NT].reshape(nblk, P).T.astype(np.float32)

    return dict(bpt=bpt, nblk=nblk, perm_pos=perm_pos, lut=lut,
                gsrc=gsrc, grel=grel, gseg=gseg)


def _build(bpt, reps=1):
    nblk = NT * bpt
    nc = bacc.Bacc("TRN2", target_bir_lowering=False, debug=False, num_devices=W)

    # ---- per-core inputs
    gsrc = nc.declare_dram_parameter("gsrc", [P, nblk], I32, isOutput=False)
    gseg = nc.declare_dram_parameter("gseg", [P, nblk], F32, isOutput=False)
    gsegT = nc.declare_dram_parameter("gsegT", [nblk, P], BF16, isOutput=False)
    relE = nc.declare_dram_parameter("relE", [nblk * P, IN], BF16, isOutput=False)
    xT_slot = nc.declare_dram_parameter("xT_slot", [IN, NSLOT], BF16,
                                        isOutput=False)
    xT_own = nc.declare_dram_parameter("xT_own", [IN, SHARD], BF16,
                                       isOutput=False)
    # ---- replicated inputs
    wl1 = nc.declare_dram_parameter("wl1", [IN, HC1], F32, isOutput=False)
    wr1 = nc.declare_dram_parameter("wr1", [IN, HC1], F32, isOutput=False)
    we1 = nc.declare_dram_parameter("we1", [IN, HC1], F32, isOutput=False)
    att1f = nc.declare_dram_parameter("att1f", [1, HC1], F32, isOutput=False)
    eb1 = nc.declare_dram_parameter("eb1", [1, HC1], F32, isOutput=False)
    ob1 = nc.declare_dram_parameter("ob1", [1, HC1], F32, isOutput=False)
    wl2 = nc.declare_dram_parameter("wl2", [HC1, HC2], F32, isOutput=False)
    wr2 = nc.declare_dram_parameter("wr2", [HC1, HC2], F32, isOutput=False)
    we2 = nc.declare_dram_parameter("we2", [IN, HC2], F32, isOutput=False)
    att2f = nc.declare_dram_parameter("att2f", [1, HC2], F32, isOutput=False)
    eb2 = nc.declare_dram_parameter("eb2", [1, HC2], F32, isOutput=False)
    ob2 = nc.declare_dram_parameter("ob2", [1, OUT], F32, isOutput=False)
    out_p = nc.declare_dram_parameter("out", [SHARD, OUT], F32, isOutput=True)
    import os as _os
    _DBG = _os.environ.get("GAT_DEBUG", "0") == "1"
    if _DBG:
        dbgV = nc.declare_dram_parameter("dbgV", [P, HC1], BF16, isOutput=True)
        dbgMf = nc.declare_dram_parameter("dbgMf", [P, HC1], BF16, isOutput=True)
        dbglg = nc.declare_dram_parameter("dbglg", [P, 3 * H], F32, isOutput=True)
        dbgwf = nc.declare_dram_parameter("dbgwf", [P, HC1], BF16, isOutput=True)
        dbgRv = nc.declare_dram_parameter("dbgRv", [P, HC1], BF16, isOutput=True)
        dbgQ = nc.declare_dram_parameter("dbgQ", [P, P], BF16, isOutput=True)
        dbgh = nc.declare_dram_parameter("dbgh", [P, HC1], BF16, isOutput=True)
        dbgxr = nc.declare_dram_parameter("dbgxr", [P, HC1], BF16, isOutput=True)

    # ---- internal DRAM
    xl1_full = nc.dram_tensor("xl1_full", [NSLOT, HC1], BF16)
    h_shard = nc.dram_tensor("h_shard", [SHARD, HC1], BF16)
    xl2_shard = [nc.dram_tensor(f"xl2_shard{k}", [CHUNK_TILES[k] * P, HC2],
                                 BF16) for k in range(NCHUNK)]
    xl2_full = nc.dram_tensor("xl2_full", [NSLOT, HC2], BF16, addr_space="Shared")

    RG = [list(range(W))]
    IOA = bass.IndirectOffsetOnAxis
    ACTF = mybir.ActivationFunctionType
    ALU = mybir.AluOpType
    K1 = 3 if bpt % 3 == 0 else 1       # superblock width, layer 1
    K2 = 3 if bpt % 3 == 0 else 2       # superblock width, layer 2

    def sb_splits(K):
        sp, j = [], 0
        while j < bpt:
            kk = min(K, bpt - j)
            sp.append((j, kk))
            j += kk
        return sp

    with tile.TileContext(nc) as tc:
        with tc.tile_pool(name="const", bufs=1) as cp:
            ident = cp.tile([P, P], BF16, tag="ident")
            make_identity(nc, ident[:])
            iotaF = cp.tile([P, P], F32, tag="iotaF")
            nc.gpsimd.iota(iotaF[:], pattern=[[1, P]], base=0,
                           channel_multiplier=0,
                           allow_small_or_imprecise_dtypes=True)
            iotaP = cp.tile([P, 1], F32, tag="iotaP")
            nc.gpsimd.iota(iotaP[:], pattern=[[1, 1]], base=0,
                           channel_multiplier=1,
                           allow_small_or_imprecise_dtypes=True)
            wl1b = cp.tile([IN, HC1], BF16, tag="wl1b")
            nc.gpsimd.dma_start(out=wl1b[:], in_=wl1[:])
            wr1b = cp.tile([IN, HC1], BF16, tag="wr1b")
            nc.gpsimd.dma_start(out=wr1b[:], in_=wr1[:])
            we1b = cp.tile([IN, HC1], BF16, tag="we1b")
            nc.gpsimd.dma_start(out=we1b[:], in_=we1[:])
            we2b = cp.tile([IN, HC2], BF16, tag="we2b")
            nc.gpsimd.dma_start(out=we2b[:], in_=we2[:])
            wl2b, wr2b = [], []
            for k in range(2):
                wl2bk = cp.tile([P, HC2], BF16, tag=f"wl2b{k}")
                nc.gpsimd.dma_start(out=wl2bk[:], in_=wl2[k * P:(k + 1) * P, :])
                wl2b.append(wl2bk)
                wr2bk = cp.tile([P, HC2], BF16, tag=f"wr2b{k}")
                nc.gpsimd.dma_start(out=wr2bk[:], in_=wr2[k * P:(k + 1) * P, :])
                wr2b.append(wr2bk)
            attB1 = cp.tile([P, K1 * HC1], BF16, tag="attB1")
            for j in range(K1):
                nc.gpsimd.dma_start(out=attB1[:, j * HC1:(j + 1) * HC1],
                                    in_=att1f[:].to_broadcast([P, HC1]))
            attB2 = cp.tile([P, K2 * HC2], BF16, tag="attB2")
            for j in range(K2):
                nc.gpsimd.dma_start(out=attB2[:, j * HC2:(j + 1) * HC2],
                                    in_=att2f[:].to_broadcast([P, HC2]))
            eb1B = cp.tile([P, HC1], F32, tag="eb1B")
            nc.sync.dma_start(out=eb1B[:], in_=eb1[:].to_broadcast([P, HC1]))
            ob1B = cp.tile([P, HC1], BF16, tag="ob1B")
            nc.gpsimd.dma_start(out=ob1B[:], in_=ob1[:].to_broadcast([P, HC1]))
            eb2B = cp.tile([P, HC2], F32, tag="eb2B")
            nc.sync.dma_start(out=eb2B[:], in_=eb2[:].to_broadcast([P, HC2]))
            ob2B = cp.tile([P, OUT], F32, tag="ob2B")
            nc.sync.dma_start(out=ob2B[:], in_=ob2[:].to_broadcast([P, OUT]))
            gsrc_t = cp.tile([P, nblk], I32, tag="gsrc_t")
            nc.sync.dma_start(out=gsrc_t[:], in_=gsrc[:])
            gseg_t = cp.tile([P, nblk], F32, tag="gseg_t")
            nc.sync.dma_start(out=gseg_t[:], in_=gseg[:])
            xr1_loc = cp.tile([P, NT * HC1], BF16, tag="xr1_loc")
            xr2_loc = cp.tile([P, NT * HC2], BF16, tag="xr2_loc")

            for _rep in range(reps):
                # ======== phase A: e-tables + xr1 + xl1_full ========
                with (
                    tc.tile_pool(name="pa_w", bufs=4) as wp,
                    tc.tile_pool(name="pa_x", bufs=1) as xp,
                    tc.tile_pool(name="pa_p", bufs=4, space="PSUM") as pp,
                ):
                    # own shard: xr1 (kept in SBUF); xl1 computed LOCALLY
                    # for all 392 tiles from a transposed x staged on host
                    # (no collective at all for layer 1).
                    xTo = xp.tile([IN, SHARD], BF16, tag="xTo")
                    nc.sync.dma_start(out=xTo[:], in_=xT_own[:])
                    for t in range(NT):
                        psR = pp.tile([P, HC1], F32, tag="psR")
                        nc.tensor.matmul(psR[:],
                                         lhsT=xTo[:, t * P:(t + 1) * P],
                                         rhs=wr1b[:], start=True, stop=True)
                        nc.vector.tensor_tensor(
                            out=xr1_loc[:, t * HC1:(t + 1) * HC1],
                            in0=psR[:], in1=eb1B[:], op=ALU.add)
                    xTs = xp.tile([IN, NSLOT], BF16, tag="xTs")
                    nc.sync.dma_start(out=xTs[:], in_=xT_slot[:])
                    for g in range(TILES):
                        r0 = _row0_of_tile(g)
                        psL = pp.tile([P, HC1], F32, tag="psL")
                        nc.tensor.matmul(psL[:], lhsT=xTs[:, r0:r0 + P],
                                         rhs=wl1b[:], start=True, stop=True)
                        xls = wp.tile([P, HC1], BF16, tag="xls")
                        if g % 2 == 0:
                            nc.scalar.activation(xls[:], psL[:], ACTF.Copy)
                        else:
                            nc.vector.tensor_copy(xls[:], psL[:])
                        if g % 2 == 0:
                            nc.scalar.dma_start(
                                out=xl1_full[r0:r0 + P, :], in_=xls[:])
                        else:
                            nc.sync.dma_start(
                                out=xl1_full[r0:r0 + P, :], in_=xls[:])

                # ======== layer 1 edge loop (+ fused layer-2 transforms) ====
                with (
                    tc.tile_pool(name="l1_w", bufs=4) as wp,
                    tc.tile_pool(name="l1_p", bufs=2, space="PSUM") as pp,
                    tc.tile_pool(name="l1_a", bufs=2, space="PSUM") as pa,
                ):
                    for t in range(NT):
                        b0 = t * bpt
                        relT = wp.tile([P, bpt * P], BF16, tag="relT")
                        nc.sync.dma_start(
                            out=relT[:],
                            in_=relE[b0 * P:(b0 + bpt) * P, :],
                            transpose=True)

                        acc1t = pa.tile([P, HC1], F32, tag="acc1t", bufs=2)
                        accd1t = pa.tile([P, H], F32, tag="accd1t", bufs=1)
                        acc1 = acc1t[:]
                        accd1 = accd1t[:]
                        XR1 = xr1_loc[:, t * HC1:(t + 1) * HC1]

                        for (j0, kk) in sb_splits(K1):
                            sfx = "" if kk == K1 else f"_{kk}"
                            V1 = wp.tile([P, kk * HC1], BF16, tag=f"V1{sfx}")
                            for jj in range(kk):
                                b = b0 + j0 + jj
                                nc.gpsimd.indirect_dma_start(
                                    out=V1[:, jj * HC1:(jj + 1) * HC1],
                                    out_offset=None, in_=xl1_full[:],
                                    in_offset=IOA(ap=gsrc_t[:, b:b + 1],
                                                  axis=0))
                            Qt = wp.tile([P, kk * P], BF16, tag=f"Qt{sfx}")
                            nc.vector.tensor_tensor(
                                out=Qt[:].rearrange("p (k q) -> p k q", q=P),
                                in0=iotaF[:].rearrange("p (o q) -> p o q", o=1)
                                .to_broadcast([P, kk, P]),
                                in1=gseg_t[:, b0 + j0:b0 + j0 + kk]
                                .rearrange("p (k o) -> p k o", o=1)
                                .to_broadcast([P, kk, P]),
                                op=ALU.is_equal)
                            segB = wp.tile([P, kk * P], BF16, tag=f"segB{sfx}")
                            nc.scalar.dma_start(
                                out=segB[:],
                                in_=gsegT[b0 + j0:b0 + j0 + kk, :]
                                .rearrange("(o k) p -> o (k p)", o=1)
                                .to_broadcast([P, kk * P]))
                            Pbt = wp.tile([P, kk * P], BF16, tag=f"Pbt{sfx}")
                            nc.vector.tensor_scalar(
                                out=Pbt[:], in0=segB[:], scalar1=iotaP[:],
                                scalar2=None, op0=ALU.is_equal)
                            psM = pp.tile([P, kk * HC1], F32, tag=f"psM{sfx}", bufs=2)
                            for jj in range(kk):
                                j = j0 + jj
                                nc.tensor.matmul(
                                    psM[:, jj * HC1:(jj + 1) * HC1],
                                    lhsT=Pbt[:, jj * P:(jj + 1) * P],
                                    rhs=XR1, start=True, stop=False)
                                nc.tensor.matmul(
                                    psM[:, jj * HC1:(jj + 1) * HC1],
                                    lhsT=relT[:, j * P:(j + 1) * P],
                                    rhs=we1b[:], start=False, stop=False)
                                nc.tensor.matmul(
                                    psM[:, jj * HC1:(jj + 1) * HC1],
                                    lhsT=ident[:],
                                    rhs=V1[:, jj * HC1:(jj + 1) * HC1],
                                    start=False, stop=True)
                            Mr = wp.tile([P, kk * HC1], BF16, tag=f"Mr{sfx}")
                            nc.scalar.activation(Mr[:], psM[:], ACTF.Prelu,
                                                 alpha=0.2)
                            Tm = wp.tile([P, kk * HC1], BF16, tag=f"Tm{sfx}")
                            nc.vector.tensor_tensor(
                                out=Tm[:], in0=Mr[:],
                                in1=attB1[:, 0:kk * HC1], op=ALU.mult)
                            logit = wp.tile([P, kk * H], F32, tag=f"lg{sfx}")
                            nc.vector.tensor_reduce(
                                out=logit[:],
                                in_=Tm[:].rearrange("p (q c) -> p q c", c=HID),
                                axis=mybir.AxisListType.X, op=ALU.add)
                            esc = wp.tile([P, kk * H], BF16, tag=f"esc{sfx}")
                            nc.scalar.activation(esc[:], logit[:], ACTF.Exp)
                            Rv = wp.tile([P, kk * HC1], BF16, tag=f"Rv{sfx}")
                            nc.vector.tensor_tensor(
                                out=Rv[:].rearrange("p (k h c) -> p k h c",
                                                    h=H, c=HID),
                                in0=V1[:].rearrange("p (k h c) -> p k h c",
                                                    h=H, c=HID),
                                in1=esc[:].rearrange("p (k h) -> p k h", h=H)
                                .rearrange("p k (h o) -> p k h o", o=1)
                                .to_broadcast([P, kk, H, HID]),
                                op=ALU.mult)
                            for jj in range(kk):
                                j = j0 + jj
                                nc.tensor.matmul(
                                    acc1, lhsT=Qt[:, jj * P:(jj + 1) * P],
                                    rhs=Rv[:, jj * HC1:(jj + 1) * HC1],
                                    start=(j == 0), stop=(j == bpt - 1))
                                nc.tensor.matmul(
                                    accd1, lhsT=Qt[:, jj * P:(jj + 1) * P],
                                    rhs=esc[:, jj * H:(jj + 1) * H],
                                    start=(j == 0), stop=(j == bpt - 1))

                        # epilogue: h, then xl2/xr2 transforms for this tile
                        dn1 = wp.tile([P, H], F32, tag="dn1")
                        nc.vector.tensor_scalar_add(dn1[:], accd1, 1e-20)
                        rec = wp.tile([P, H], F32, tag="rec")
                        nc.vector.reciprocal(rec[:], dn1[:])
                        htmp = wp.tile([P, HC1], BF16, tag="htmp")
                        for hh in range(H):
                            nc.scalar.activation(
                                htmp[:, hh * HID:(hh + 1) * HID],
                                acc1[:, hh * HID:(hh + 1) * HID],
                                ACTF.Copy, scale=rec[:, hh:hh + 1])
                        hsb = wp.tile([P, HC1], BF16, tag="hsb")
                        nc.vector.tensor_tensor(out=hsb[:], in0=htmp[:],
                                                in1=ob1B[:], op=ALU.add)
                        if _DBG and t == 0:
                            nc.sync.dma_start(out=dbgh[:], in_=hsb[:])
                            nc.sync.dma_start(
                                out=dbgxr[:], in_=xr1_loc[:, 0:HC1])
                        nc.sync.dma_start(out=h_shard[t * P:(t + 1) * P, :],
                                          in_=hsb[:])
                        hTt = wp.tile([P, HC1], BF16, tag="hTt")
                        for k in range(2):
                            nc.sync.dma_start(
                                out=hTt[:, k * P:(k + 1) * P],
                                in_=h_shard[t * P:(t + 1) * P,
                                            k * P:(k + 1) * P],
                                transpose=True)
                        psC2 = pp.tile([P, HC2], F32, tag="psC", bufs=1)
                        for k in range(2):
                            nc.tensor.matmul(psC2[:, 0:HC2],
                                             lhsT=hTt[:, k * P:(k + 1) * P],
                                             rhs=wl2b[k][:],
                                             start=(k == 0), stop=(k == 1))
                        xl2sb = wp.tile([P, HC2], BF16, tag="xl2sb")
                        nc.scalar.activation(xl2sb[:], psC2[:, 0:HC2], ACTF.Copy)
                        ck = _chunk_of_tile(t)
                        cr = (t - CHUNK_T0[ck]) * P
                        nc.sync.dma_start(
                            out=xl2_shard[ck][cr:cr + P, :], in_=xl2sb[:])
                        psC3 = pp.tile([P, HC2], F32, tag="psC", bufs=1)
                        for k in range(2):
                            nc.tensor.matmul(psC3[:, 0:HC2],
                                             lhsT=hTt[:, k * P:(k + 1) * P],
                                             rhs=wr2b[k][:],
                                             start=(k == 0), stop=(k == 1))
                        nc.vector.tensor_tensor(
                            out=xr2_loc[:, t * HC2:(t + 1) * HC2],
                            in0=psC3[:, 0:HC2], in1=eb2B[:], op=ALU.add)

                        # chunked AllGather as soon as a chunk's tiles are done
                        for k in range(NCHUNK):
                            if t == CHUNK_T0[k] + CHUNK_TILES[k] - 1:
                                o0 = CHUNK_BASE[k]
                                o1 = o0 + W * CHUNK_TILES[k] * P
                                nc.gpsimd.collective_compute(
                                    "AllGather", ALU.bypass,
                                    ins=[xl2_shard[k][:]],
                                    outs=[xl2_full[o0:o1, :]],
                                    replica_groups=RG)

                # ======== layer 2 edge loop ========
                with (
                    tc.tile_pool(name="l2_w", bufs=4) as wp,
                    tc.tile_pool(name="l2_p", bufs=2, space="PSUM") as pp,
                    tc.tile_pool(name="l2_a", bufs=1, space="PSUM") as pa,
                ):
                    for t in range(NT):
                        b0 = t * bpt
                        relT = wp.tile([P, bpt * P], BF16, tag="relT2")
                        nc.sync.dma_start(
                            out=relT[:],
                            in_=relE[b0 * P:(b0 + bpt) * P, :],
                            transpose=True)

                        acc2t = pa.tile([P, HC2], F32, tag="acc2t", bufs=2)
                        accd2t = pa.tile([P, H], F32, tag="accd2t", bufs=2)
                        acc2 = acc2t[:]
                        accd2 = accd2t[:]
                        XR2 = xr2_loc[:, t * HC2:(t + 1) * HC2]

                        for (j0, kk) in sb_splits(K2):
                            sfx = "" if kk == K2 else f"_{kk}"
                            V2 = wp.tile([P, kk * HC2], BF16, tag=f"V2{sfx}")
                            for jj in range(kk):
                                b = b0 + j0 + jj
                                nc.gpsimd.indirect_dma_start(
                                    out=V2[:, jj * HC2:(jj + 1) * HC2],
                                    out_offset=None, in_=xl2_full[:],
                                    in_offset=IOA(ap=gsrc_t[:, b:b + 1],
                                                  axis=0))
                            Qt = wp.tile([P, kk * P], BF16, tag=f"Qt{sfx}")
                            nc.vector.tensor_tensor(
                                out=Qt[:].rearrange("p (k q) -> p k q", q=P),
                                in0=iotaF[:].rearrange("p (o q) -> p o q", o=1)
                                .to_broadcast([P, kk, P]),
                                in1=gseg_t[:, b0 + j0:b0 + j0 + kk]
                                .rearrange("p (k o) -> p k o", o=1)
                                .to_broadcast([P, kk, P]),
                                op=ALU.is_equal)
                            segB = wp.tile([P, kk * P], BF16, tag=f"segB{sfx}")
                            nc.scalar.dma_start(
                                out=segB[:],
                                in_=gsegT[b0 + j0:b0 + j0 + kk, :]
                                .rearrange("(o k) p -> o (k p)", o=1)
                                .to_broadcast([P, kk * P]))
                            Pbt = wp.tile([P, kk * P], BF16, tag=f"Pbt{sfx}")
                            nc.vector.tensor_scalar(
                                out=Pbt[:], in0=segB[:], scalar1=iotaP[:],
                                scalar2=None, op0=ALU.is_equal)
                            psM = pp.tile([P, kk * HC2], F32, tag=f"psM{sfx}", bufs=1)
                            for jj in range(kk):
                                j = j0 + jj
                                nc.tensor.matmul(
                                    psM[:, jj * HC2:(jj + 1) * HC2],
                                    lhsT=Pbt[:, jj * P:(jj + 1) * P],
                                    rhs=XR2, start=True, stop=False)
                                nc.tensor.matmul(
                                    psM[:, jj * HC2:(jj + 1) * HC2],
                                    lhsT=relT[:, j * P:(j + 1) * P],
                                    rhs=we2b[:], start=False, stop=False)
                                nc.tensor.matmul(
                                    psM[:, jj * HC2:(jj + 1) * HC2],
                                    lhsT=ident[:],
                                    rhs=V2[:, jj * HC2:(jj + 1) * HC2],
                                    start=False, stop=True)
                            Mr = wp.tile([P, kk * HC2], BF16, tag=f"Mr{sfx}")
                            nc.scalar.activation(Mr[:], psM[:], ACTF.Prelu,
                                                 alpha=0.2)
                            Tm = wp.tile([P, kk * HC2], BF16, tag=f"Tm{sfx}")
                            nc.vector.tensor_tensor(
                                out=Tm[:], in0=Mr[:],
                                in1=attB2[:, 0:kk * HC2], op=ALU.mult)
                            logit = wp.tile([P, kk * H], F32, tag=f"lg{sfx}")
                            nc.vector.tensor_reduce(
                                out=logit[:],
                                in_=Tm[:].rearrange("p (q c) -> p q c", c=OUT),
                                axis=mybir.AxisListType.X, op=ALU.add)
                            esc = wp.tile([P, kk * H], BF16, tag=f"esc{sfx}")
                            nc.scalar.activation(esc[:], logit[:], ACTF.Exp)
                            Rv = wp.tile([P, kk * HC2], BF16, tag=f"Rv{sfx}")
                            nc.vector.tensor_tensor(
                                out=Rv[:].rearrange("p (k h c) -> p k h c",
                                                    h=H, c=OUT),
                                in0=V2[:].rearrange("p (k h c) -> p k h c",
                                                    h=H, c=OUT),
                                in1=esc[:].rearrange("p (k h) -> p k h", h=H)
                                .rearrange("p k (h o) -> p k h o", o=1)
                                .to_broadcast([P, kk, H, OUT]),
                                op=ALU.mult)
                            for jj in range(kk):
                                j = j0 + jj
                                nc.tensor.matmul(
                                    acc2, lhsT=Qt[:, jj * P:(jj + 1) * P],
                                    rhs=Rv[:, jj * HC2:(jj + 1) * HC2],
                                    start=(j == 0), stop=(j == bpt - 1))
                                nc.tensor.matmul(
                                    accd2, lhsT=Qt[:, jj * P:(jj + 1) * P],
                                    rhs=esc[:, jj * H:(jj + 1) * H],
                                    start=(j == 0), stop=(j == bpt - 1))

                        # epilogue: out = mean_h(acc_h/denom_h) + bias
                        dn2 = wp.tile([P, H], F32, tag="dn2")
                        nc.vector.tensor_scalar_add(dn2[:], accd2, 1e-20)
                        rec2 = wp.tile([P, H], F32, tag="rec2")
                        nc.vector.reciprocal(rec2[:], dn2[:])
                        rec4 = wp.tile([P, H], F32, tag="rec4")
                        nc.vector.tensor_scalar_mul(rec4[:], rec2[:], 0.25)
                        ho = wp.tile([P, H * OUT], F32, tag="ho")
                        for hh in range(H):
                            nc.scalar.activation(
                                ho[:, hh * OUT:(hh + 1) * OUT],
                                acc2[:, hh * OUT:(hh + 1) * OUT],
                                ACTF.Copy, scale=rec4[:, hh:hh + 1])
                        s01 = wp.tile([P, OUT], F32, tag="s01")
                        nc.vector.tensor_tensor(out=s01[:], in0=ho[:, 0:OUT],
                                                in1=ho[:, OUT:2 * OUT],
                                                op=ALU.add)
                        s23 = wp.tile([P, OUT], F32, tag="s23")
                        nc.gpsimd.tensor_tensor(out=s23[:],
                                                in0=ho[:, 2 * OUT:3 * OUT],
                                                in1=ho[:, 3 * OUT:4 * OUT],
                                                op=ALU.add)
                        s03 = wp.tile([P, OUT], F32, tag="s03")
                        nc.vector.tensor_tensor(out=s03[:], in0=s01[:],
                                                in1=s23[:], op=ALU.add)
                        osb = wp.tile([P, OUT], F32, tag="osb")
                        nc.gpsimd.tensor_tensor(out=osb[:], in0=s03[:],
                                                in1=ob2B[:], op=ALU.add)
                        nc.sync.dma_start(out=out_p[t * P:(t + 1) * P, :],
                                          in_=osb[:])

    nc.compile()
    return nc


def _make_in_maps(inp, pre):
    f32 = np.float32
    lut = pre["lut"]
    perm_pos = pre["perm_pos"]
    x = np.asarray(inp["x"], f32)
    rel_padz = np.zeros((R + 1, IN), BF)
    rel_padz[:R] = np.asarray(inp["relations"], f32).astype(BF)
    a = lambda k: np.asarray(inp[k], f32)
    rep = dict(
        wl1=a("Wl1"), wr1=a("Wr1"), we1=a("We1"),
        att1f=a("att1").reshape(1, HC1),
        eb1=(a("bl1") + a("br1")).reshape(1, HC1),
        ob1=(a("bl1") + a("bias1")).reshape(1, HC1),
        wl2=a("Wl2"), wr2=a("Wr2"), we2=a("We2"),
        att2f=a("att2").reshape(1, HC2),
        eb2=(a("bl2") + a("br2")).reshape(1, HC2),
        ob2=(a("bl2").reshape(H, OUT).mean(axis=0) + a("bias2")).reshape(1, OUT),
    )
    # x in chunk-major slot rows, staged transposed for direct matmul lhsT
    x_slot = np.zeros((NSLOT, IN), BF)      # chunk-major row order
    x_slot[lut[perm_pos]] = x.astype(BF)
    xT_slot = np.ascontiguousarray(x_slot.T)
    in_maps = []
    for c in range(W):
        m = dict(rep)
        m["gsrc"] = np.ascontiguousarray(pre["gsrc"][c])
        m["gseg"] = np.ascontiguousarray(pre["gseg"][c])
        m["gsegT"] = np.ascontiguousarray(pre["gseg"][c].T.astype(BF))
        m["relE"] = np.ascontiguousarray(
            rel_padz[np.minimum(pre["grel"][c].T.reshape(-1), R)])
        m["xT_slot"] = xT_slot
        base = c * NT * P
        m["xT_own"] = np.ascontiguousarray(x_slot[lut[base:base + SHARD]].T)
        in_maps.append(m)
    return in_maps


_CACHE = {}


def kernel(x, edge_index, relations,
           Wl1, bl1, Wr1, br1, We1, att1, bias1,
           Wl2, bl2, Wr2, br2, We2, att2, bias2, **_unused):
    x = np.asarray(x, np.float32)
    edge_index = np.asarray(edge_index)
    relations = np.asarray(relations, np.float32)

    pre = _preprocess(edge_index)
    bpt = pre["bpt"]

    if bpt not in _CACHE:
        _CACHE[bpt] = _build(bpt)
    nc = _CACHE[bpt]

    in_maps = _make_in_maps(
        dict(x=x, relations=relations, Wl1=Wl1, bl1=bl1, Wr1=Wr1, br1=br1,
             We1=We1, att1=att1, bias1=bias1, Wl2=Wl2, bl2=bl2, Wr2=Wr2,
             br2=br2, We2=We2, att2=att2, bias2=bias2), pre)

    import os
    trace = os.environ.get("GAT_TRACE", "0") == "1"
    res = run_bass_kernel_spmd(nc, in_maps, list(range(W)), trace=trace)
    global LAST_EXEC_NS, LAST_RES
    LAST_EXEC_NS = res.exec_time_ns
    LAST_RES = res
    cat = np.concatenate([res.results[c]["out"] for c in range(W)], axis=0)
    return np.ascontiguousarray(cat[pre["perm_pos"]])


if __name__ == "__main__":
    pass



# revision 10
# speedup vs baseline: 32.6423x; 1.1116x over previous
"""GATv2 2-layer encoder on 8 TRN2 NeuronCores — v4.

Destination-node sharding in a permuted "slot" space: nodes are bin-packed
into 392 tiles of 128 slots (balancing in-edge counts), 49 tiles per core.
Layer 1 needs no collective: every core computes the full xl1 table locally
from a host-transposed x (one matmul per tile, no DMA transposes); only the
layer-2 source table is AllGathered.  Per edge block, xl[src] rows come via
single-column indirect DMAs; xr[dst] and the relation embedding are added
in PSUM via one-hot / transposed-relation matmuls, and the gathered xl rows
are merged with an identity matmul so the pre-activation message never
takes a DVE pass.  exp(logit) is computed at [E,H] (not broadcast to
[E,H*C]); the alpha*xl product is one DVE broadcast multiply; the softmax
denominator accumulates into spare columns of the same PSUM bank as the
numerator.  One-hot scatter/select matrices are built on-chip with single
broadcast is_equal ops per superblock.
"""
import sys
import heapq

import numpy as np

sys.path.insert(0, "/opt/trn_rl_repo")

import ml_dtypes  # noqa: E402
import concourse.bass as bass  # noqa: E402
import concourse.tile as tile  # noqa: E402
from concourse import bacc, mybir  # noqa: E402
from concourse.bass_utils import run_bass_kernel_spmd  # noqa: E402
from concourse.masks import make_identity  # noqa: E402

N, E, R = 50000, 400000, 500
IN, HID, H, OUT = 128, 64, 4, 128
HC1, HC2 = H * HID, H * OUT  # 256, 512
W = 8            # cores
P = 128          # partitions / tile slots / edge-block size
NT = 49          # node tiles per core
TILES = W * NT   # 392
NSLOT = TILES * P  # 50176
SHARD = NT * P   # 6272 rows per core
RPAD = 512       # padded relation table rows (rows R.. are zero)

F32 = mybir.dt.float32
BF16 = mybir.dt.bfloat16
I32 = mybir.dt.int32
BF = ml_dtypes.bfloat16

# xl2 AllGather chunking (tiles per chunk); sum must be NT
CHUNK_TILES = [49]
CHUNK_T0 = np.cumsum([0] + CHUNK_TILES[:-1]).tolist()
CHUNK_BASE = np.cumsum(
    [0] + [W * ct * P for ct in CHUNK_TILES[:-1]]).tolist()
NCHUNK = len(CHUNK_TILES)


def _chunk_of_tile(t):
    for k in range(NCHUNK):
        if CHUNK_T0[k] <= t < CHUNK_T0[k] + CHUNK_TILES[k]:
            return k
    raise AssertionError


def _row0_of_tile(g):
    """First chunk-major DRAM row of global tile g in a full node table."""
    c, t = g // NT, g % NT
    k = _chunk_of_tile(t)
    return CHUNK_BASE[k] + c * CHUNK_TILES[k] * P + (t - CHUNK_T0[k]) * P


_LUT = None


def _chunkrow_lut():
    """slot (tile-major) -> chunk-major DRAM row."""
    global _LUT
    if _LUT is None:
        lut = np.empty(NSLOT, np.int64)
        for g in range(TILES):
            lut[g * P:(g + 1) * P] = _row0_of_tile(g) + np.arange(P)
        _LUT = lut
    return _LUT


def _preprocess(edge_index):
    """Self-loops, balanced node->tile binning, per-core index planes."""
    src = np.asarray(edge_index[0], dtype=np.int64)
    rel = np.asarray(edge_index[1], dtype=np.int64)
    dst = np.asarray(edge_index[2], dtype=np.int64)
    loop = np.arange(N, dtype=np.int64)
    src_f = np.concatenate([src, loop])
    dst_f = np.concatenate([dst, loop])
    rel_f = np.concatenate([rel, np.full(N, R, dtype=np.int64)])

    deg = np.bincount(dst_f, minlength=N)

    order = np.argsort(-deg, kind="stable")
    tile_of = np.empty(N, np.int64)
    slot_of = np.empty(N, np.int64)
    heap = [(0, t) for t in range(TILES)]
    heapq.heapify(heap)
    counts = np.zeros(TILES, np.int64)
    loads = np.zeros(TILES, np.int64)
    for n in order:
        while True:
            load, t = heapq.heappop(heap)
            if counts[t] < P:
                break
        tile_of[n] = t
        slot_of[n] = counts[t]
        counts[t] += 1
        loads[t] += deg[n]
        if counts[t] < P:
            heapq.heappush(heap, (loads[t], t))

    perm_pos = tile_of * P + slot_of          # node -> slot (tile-major)
    lut = _chunkrow_lut()

    bpt = max(1, int(-(-loads.max() // P)))   # blocks per tile (uniform)
    nblk = NT * bpt
    cap = bpt * P

    et = tile_of[dst_f]
    eorder = np.argsort(et, kind="stable")
    et_s = et[eorder]
    starts = np.searchsorted(et_s, np.arange(TILES))
    ends = np.searchsorted(et_s, np.arange(TILES), side="right")

    src_a = np.zeros((TILES, cap), np.int64)          # pad -> row 0
    rel_a = np.full((TILES, cap), R, np.int64)        # pad -> zero e-row
    seg_a = np.full((TILES, cap), 999, np.int64)      # pad -> no one-hot
    for t in range(TILES):
        idx = eorder[starts[t]:ends[t]]
        k = idx.shape[0]
        src_a[t, :k] = lut[perm_pos[src_f[idx]]]
        rel_a[t, :k] = rel_f[idx]
        seg_a[t, :k] = slot_of[dst_f[idx]]

    gsrc = np.zeros((W, P, nblk), np.int32)
    grel = np.zeros((W, P, nblk), np.int32)
    gseg = np.zeros((W, P, nblk), np.float32)
    for c in range(W):
        g0 = c * NT
        gsrc[c] = src_a[g0:g0 + NT].reshape(nblk, P).T
        grel[c] = rel_a[g0:g0 + NT].reshape(nblk, P).T
        gseg[c] = seg_a[g0:g0 + NT].reshape(nblk, P).T.astype(np.float32)

    return dict(bpt=bpt, nblk=nblk, perm_pos=perm_pos, lut=lut,
                gsrc=gsrc, grel=grel, gseg=gseg)


def _build(bpt, reps=1):
    nblk = NT * bpt
    nc = bacc.Bacc("TRN2", target_bir_lowering=False, debug=False, num_devices=W)

    # ---- per-core inputs
    gsrc = nc.declare_dram_parameter("gsrc", [P, nblk], I32, isOutput=False)
    gseg = nc.declare_dram_parameter("gseg", [P, nblk], F32, isOutput=False)
    gsegT = nc.declare_dram_parameter("gsegT", [nblk, P], BF16, isOutput=False)
    relE = nc.declare_dram_parameter("relE", [nblk * P, IN], BF16, isOutput=False)
    xT_slot = nc.declare_dram_parameter("xT_slot", [IN, NSLOT], BF16,
                                        isOutput=False)
    xT_own = nc.declare_dram_parameter("xT_own", [IN, SHARD], BF16,
                                       isOutput=False)
    # ---- replicated inputs
    wl1 = nc.declare_dram_parameter("wl1", [IN, HC1], F32, isOutput=False)
    wr1 = nc.declare_dram_parameter("wr1", [IN, HC1], F32, isOutput=False)
    we1 = nc.declare_dram_parameter("we1", [IN, HC1], F32, isOutput=False)
    att1f = nc.declare_dram_parameter("att1f", [1, HC1], F32, isOutput=False)
    eb1 = nc.declare_dram_parameter("eb1", [1, HC1], F32, isOutput=False)
    ob1 = nc.declare_dram_parameter("ob1", [1, HC1], F32, isOutput=False)
    wl2 = nc.declare_dram_parameter("wl2", [HC1, HC2], F32, isOutput=False)
    wr2 = nc.declare_dram_parameter("wr2", [HC1, HC2], F32, isOutput=False)
    we2 = nc.declare_dram_parameter("we2", [IN, HC2], F32, isOutput=False)
    att2f = nc.declare_dram_parameter("att2f", [1, HC2], F32, isOutput=False)
    eb2 = nc.declare_dram_parameter("eb2", [1, HC2], F32, isOutput=False)
    ob2 = nc.declare_dram_parameter("ob2", [1, OUT], F32, isOutput=False)
    out_p = nc.declare_dram_parameter("out", [SHARD, OUT], F32, isOutput=True)
    import os as _os
    _DBG = _os.environ.get("GAT_DEBUG", "0") == "1"
    if _DBG:
        dbgV = nc.declare_dram_parameter("dbgV", [P, HC1], BF16, isOutput=True)
        dbgMf = nc.declare_dram_parameter("dbgMf", [P, HC1], BF16, isOutput=True)
        dbglg = nc.declare_dram_parameter("dbglg", [P, 3 * H], F32, isOutput=True)
        dbgwf = nc.declare_dram_parameter("dbgwf", [P, HC1], BF16, isOutput=True)
        dbgRv = nc.declare_dram_parameter("dbgRv", [P, HC1], BF16, isOutput=True)
        dbgQ = nc.declare_dram_parameter("dbgQ", [P, P], BF16, isOutput=True)
        dbgh = nc.declare_dram_parameter("dbgh", [P, HC1], BF16, isOutput=True)
        dbgxr = nc.declare_dram_parameter("dbgxr", [P, HC1], BF16, isOutput=True)

    # ---- internal DRAM
    xl1_full = nc.dram_tensor("xl1_full", [NSLOT, HC1], BF16)
    h_shard = nc.dram_tensor("h_shard", [SHARD, HC1], BF16)
    xl2_shard = [nc.dram_tensor(f"xl2_shard{k}", [CHUNK_TILES[k] * P, HC2],
                                 BF16) for k in range(NCHUNK)]
    xl2_full = nc.dram_tensor("xl2_full", [NSLOT, HC2], BF16, addr_space="Shared")

    RG = [list(range(W))]
    IOA = bass.IndirectOffsetOnAxis
    ACTF = mybir.ActivationFunctionType
    ALU = mybir.AluOpType
    K1 = 3 if bpt % 3 == 0 else 1       # superblock width, layer 1
    K2 = 3 if bpt % 3 == 0 else 2       # superblock width, layer 2

    def sb_splits(K):
        sp, j = [], 0
        while j < bpt:
            kk = min(K, bpt - j)
            sp.append((j, kk))
            j += kk
        return sp

    with tile.TileContext(nc) as tc:
        with tc.tile_pool(name="const", bufs=1) as cp:
            ident = cp.tile([P, P], BF16, tag="ident")
            make_identity(nc, ident[:])
            iotaF = cp.tile([P, P], F32, tag="iotaF")
            nc.gpsimd.iota(iotaF[:], pattern=[[1, P]], base=0,
                           channel_multiplier=0,
                           allow_small_or_imprecise_dtypes=True)
            iotaP = cp.tile([P, 1], F32, tag="iotaP")
            nc.gpsimd.iota(iotaP[:], pattern=[[1, 1]], base=0,
                           channel_multiplier=1,
                           allow_small_or_imprecise_dtypes=True)
            wl1b = cp.tile([IN, HC1], BF16, tag="wl1b")
            nc.gpsimd.dma_start(out=wl1b[:], in_=wl1[:])
            wr1b = cp.tile([IN, HC1], BF16, tag="wr1b")
            nc.gpsimd.dma_start(out=wr1b[:], in_=wr1[:])
            we1b = cp.tile([IN, HC1], BF16, tag="we1b")
            nc.gpsimd.dma_start(out=we1b[:], in_=we1[:])
            we2b = cp.tile([IN, HC2], BF16, tag="we2b")
            nc.gpsimd.dma_start(out=we2b[:], in_=we2[:])
            wl2b, wr2b = [], []
            for k in range(2):
                wl2bk = cp.tile([P, HC2], BF16, tag=f"wl2b{k}")
                nc.gpsimd.dma_start(out=wl2bk[:], in_=wl2[k * P:(k + 1) * P, :])
                wl2b.append(wl2bk)
                wr2bk = cp.tile([P, HC2], BF16, tag=f"wr2b{k}")
                nc.gpsimd.dma_start(out=wr2bk[:], in_=wr2[k * P:(k + 1) * P, :])
                wr2b.append(wr2bk)
            attB1 = cp.tile([P, K1 * HC1], BF16, tag="attB1")
            for j in range(K1):
                nc.gpsimd.dma_start(out=attB1[:, j * HC1:(j + 1) * HC1],
                                    in_=att1f[:].to_broadcast([P, HC1]))
            attB2 = cp.tile([P, K2 * HC2], BF16, tag="attB2")
            for j in range(K2):
                nc.gpsimd.dma_start(out=attB2[:, j * HC2:(j + 1) * HC2],
                                    in_=att2f[:].to_broadcast([P, HC2]))
            eb1B = cp.tile([P, HC1], F32, tag="eb1B")
            nc.sync.dma_start(out=eb1B[:], in_=eb1[:].to_broadcast([P, HC1]))
            ob1B = cp.tile([P, HC1], BF16, tag="ob1B")
            nc.gpsimd.dma_start(out=ob1B[:], in_=ob1[:].to_broadcast([P, HC1]))
            eb2B = cp.tile([P, HC2], F32, tag="eb2B")
            nc.sync.dma_start(out=eb2B[:], in_=eb2[:].to_broadcast([P, HC2]))
            ob2B = cp.tile([P, OUT], F32, tag="ob2B")
            nc.sync.dma_start(out=ob2B[:], in_=ob2[:].to_broadcast([P, OUT]))
            gsrc_t = cp.tile([P, nblk], I32, tag="gsrc_t")
            nc.sync.dma_start(out=gsrc_t[:], in_=gsrc[:])
            gseg_t = cp.tile([P, nblk], F32, tag="gseg_t")
            nc.sync.dma_start(out=gseg_t[:], in_=gseg[:])
            xr1_loc = cp.tile([P, NT * HC1], BF16, tag="xr1_loc")
            xr2_loc = cp.tile([P, NT * HC2], BF16, tag="xr2_loc")

            for _rep in range(reps):
                # ======== phase A: e-tables + xr1 + xl1_full ========
                with (
                    tc.tile_pool(name="pa_w", bufs=4) as wp,
                    tc.tile_pool(name="pa_x", bufs=1) as xp,
                    tc.tile_pool(name="pa_p", bufs=4, space="PSUM") as pp,
                ):
                    # own shard: xr1 (kept in SBUF); xl1 computed LOCALLY
                    # for all 392 tiles from a transposed x staged on host
                    # (no collective at all for layer 1).
                    xTo = xp.tile([IN, SHARD], BF16, tag="xTo")
                    nc.sync.dma_start(out=xTo[:], in_=xT_own[:])
                    for t in range(NT):
                        psR = pp.tile([P, HC1], F32, tag="psR")
                        nc.tensor.matmul(psR[:],
                                         lhsT=xTo[:, t * P:(t + 1) * P],
                                         rhs=wr1b[:], start=True, stop=True)
                        nc.vector.tensor_tensor(
                            out=xr1_loc[:, t * HC1:(t + 1) * HC1],
                            in0=psR[:], in1=eb1B[:], op=ALU.add)
                    xTs = xp.tile([IN, NSLOT], BF16, tag="xTs")
                    nc.sync.dma_start(out=xTs[:], in_=xT_slot[:])
                    for g in range(TILES):
                        r0 = _row0_of_tile(g)
                        psL = pp.tile([P, HC1], F32, tag="psL")
                        nc.tensor.matmul(psL[:], lhsT=xTs[:, r0:r0 + P],
                                         rhs=wl1b[:], start=True, stop=True)
                        xls = wp.tile([P, HC1], BF16, tag="xls")
                        if g % 2 == 0:
                            nc.scalar.activation(xls[:], psL[:], ACTF.Copy)
                        else:
                            nc.vector.tensor_copy(xls[:], psL[:])
                        if g % 2 == 0:
                            nc.scalar.dma_start(
                                out=xl1_full[r0:r0 + P, :], in_=xls[:])
                        else:
                            nc.sync.dma_start(
                                out=xl1_full[r0:r0 + P, :], in_=xls[:])

                # ======== layer 1 edge loop (+ fused layer-2 transforms) ====
                with (
                    tc.tile_pool(name="l1_w", bufs=4) as wp,
                    tc.tile_pool(name="l1_p", bufs=2, space="PSUM") as pp,
                    tc.tile_pool(name="l1_a", bufs=2, space="PSUM") as pa,
                ):
                    for t in range(NT):
                        b0 = t * bpt
                        relT = wp.tile([P, bpt * P], BF16, tag="relT")
                        nc.sync.dma_start(
                            out=relT[:],
                            in_=relE[b0 * P:(b0 + bpt) * P, :],
                            transpose=True)

                        acc1t = pa.tile([P, HC1], F32, tag="acc1t", bufs=2)
                        accd1t = pa.tile([P, H], F32, tag="accd1t", bufs=1)
                        acc1 = acc1t[:]
                        accd1 = accd1t[:]
                        XR1 = xr1_loc[:, t * HC1:(t + 1) * HC1]

                        for (j0, kk) in sb_splits(K1):
                            sfx = "" if kk == K1 else f"_{kk}"
                            V1 = wp.tile([P, kk * HC1], BF16, tag=f"V1{sfx}")
                            for jj in range(kk):
                                b = b0 + j0 + jj
                                nc.gpsimd.indirect_dma_start(
                                    out=V1[:, jj * HC1:(jj + 1) * HC1],
                                    out_offset=None, in_=xl1_full[:],
                                    in_offset=IOA(ap=gsrc_t[:, b:b + 1],
                                                  axis=0))
                            Qt = wp.tile([P, kk * P], BF16, tag=f"Qt{sfx}")
                            nc.vector.tensor_tensor(
                                out=Qt[:].rearrange("p (k q) -> p k q", q=P),
                                in0=iotaF[:].rearrange("p (o q) -> p o q", o=1)
                                .to_broadcast([P, kk, P]),
                                in1=gseg_t[:, b0 + j0:b0 + j0 + kk]
                                .rearrange("p (k o) -> p k o", o=1)
                                .to_broadcast([P, kk, P]),
                                op=ALU.is_equal)
                            segB = wp.tile([P, kk * P], BF16, tag=f"segB{sfx}")
                            nc.scalar.dma_start(
                                out=segB[:],
                                in_=gsegT[b0 + j0:b0 + j0 + kk, :]
                                .rearrange("(o k) p -> o (k p)", o=1)
                                .to_broadcast([P, kk * P]))
                            Pbt = wp.tile([P, kk * P], BF16, tag=f"Pbt{sfx}")
                            nc.vector.tensor_scalar(
                                out=Pbt[:], in0=segB[:], scalar1=iotaP[:],
                                scalar2=None, op0=ALU.is_equal)
                            psM = pp.tile([P, kk * HC1], F32, tag=f"psM{sfx}", bufs=2)
                            for jj in range(kk):
                                j = j0 + jj
                                nc.tensor.matmul(
                                    psM[:, jj * HC1:(jj + 1) * HC1],
                                    lhsT=Pbt[:, jj * P:(jj + 1) * P],
                                    rhs=XR1, start=True, stop=False)
                                nc.tensor.matmul(
                                    psM[:, jj * HC1:(jj + 1) * HC1],
                                    lhsT=relT[:, j * P:(j + 1) * P],
                                    rhs=we1b[:], start=False, stop=False)
                                nc.tensor.matmul(
                                    psM[:, jj * HC1:(jj + 1) * HC1],
                                    lhsT=ident[:],
                                    rhs=V1[:, jj * HC1:(jj + 1) * HC1],
                                    start=False, stop=True)
                            Mr = wp.tile([P, kk * HC1], BF16, tag=f"Mr{sfx}")
                            nc.scalar.activation(Mr[:], psM[:], ACTF.Prelu,
                                                 alpha=0.2)
                            Tm = wp.tile([P, kk * HC1], BF16, tag=f"Tm{sfx}")
                            nc.vector.tensor_tensor(
                                out=Tm[:], in0=Mr[:],
                                in1=attB1[:, 0:kk * HC1], op=ALU.mult)
                            logit = wp.tile([P, kk * H], F32, tag=f"lg{sfx}")
                            nc.vector.tensor_reduce(
                                out=logit[:],
                                in_=Tm[:].rearrange("p (q c) -> p q c", c=HID),
                                axis=mybir.AxisListType.X, op=ALU.add)
                            esc = wp.tile([P, kk * H], BF16, tag=f"esc{sfx}")
                            nc.scalar.activation(esc[:], logit[:], ACTF.Exp)
                            Rv = wp.tile([P, kk * HC1], BF16, tag=f"Rv{sfx}")
                            nc.vector.tensor_tensor(
                                out=Rv[:].rearrange("p (k h c) -> p k h c",
                                                    h=H, c=HID),
                                in0=V1[:].rearrange("p (k h c) -> p k h c",
                                                    h=H, c=HID),
                                in1=esc[:].rearrange("p (k h) -> p k h", h=H)
                                .rearrange("p k (h o) -> p k h o", o=1)
                                .to_broadcast([P, kk, H, HID]),
                                op=ALU.mult)
                            for jj in range(kk):
                                j = j0 + jj
                                nc.tensor.matmul(
                                    acc1, lhsT=Qt[:, jj * P:(jj + 1) * P],
                                    rhs=Rv[:, jj * HC1:(jj + 1) * HC1],
                                    start=(j == 0), stop=(j == bpt - 1))
                                nc.tensor.matmul(
                                    accd1, lhsT=Qt[:, jj * P:(jj + 1) * P],
                                    rhs=esc[:, jj * H:(jj + 1) * H],
                                    start=(j == 0), stop=(j == bpt - 1))

                        # epilogue: h, then xl2/xr2 transforms for this tile
                        dn1 = wp.tile([P, H], F32, tag="dn1")
                        nc.vector.tensor_scalar_add(dn1[:], accd1, 1e-20)
                        rec = wp.tile([P, H], F32, tag="rec")
                        nc.vector.reciprocal(rec[:], dn1[:])
                        htmp = wp.tile([P, HC1], BF16, tag="htmp")
                        for hh in range(H):
                            nc.scalar.activation(
                                htmp[:, hh * HID:(hh + 1) * HID],
                                acc1[:, hh * HID:(hh + 1) * HID],
                                ACTF.Copy, scale=rec[:, hh:hh + 1])
                        hsb = wp.tile([P, HC1], BF16, tag="hsb")
                        nc.vector.tensor_tensor(out=hsb[:], in0=htmp[:],
                                                in1=ob1B[:], op=ALU.add)
                        if _DBG and t == 0:
                            nc.sync.dma_start(out=dbgh[:], in_=hsb[:])
                            nc.sync.dma_start(
                                out=dbgxr[:], in_=xr1_loc[:, 0:HC1])
                        nc.sync.dma_start(out=h_shard[t * P:(t + 1) * P, :],
                                          in_=hsb[:])
                        hTt = wp.tile([P, HC1], BF16, tag="hTt")
                        for k in range(2):
                            nc.sync.dma_start(
                                out=hTt[:, k * P:(k + 1) * P],
                                in_=h_shard[t * P:(t + 1) * P,
                                            k * P:(k + 1) * P],
                                transpose=True)
                        psC2 = pp.tile([P, HC2], F32, tag="psC", bufs=1)
                        for k in range(2):
                            nc.tensor.matmul(psC2[:, 0:HC2],
                                             lhsT=hTt[:, k * P:(k + 1) * P],
                                             rhs=wl2b[k][:],
                                             start=(k == 0), stop=(k == 1))
                        xl2sb = wp.tile([P, HC2], BF16, tag="xl2sb")
                        nc.scalar.activation(xl2sb[:], psC2[:, 0:HC2], ACTF.Copy)
                        ck = _chunk_of_tile(t)
                        cr = (t - CHUNK_T0[ck]) * P
                        nc.sync.dma_start(
                            out=xl2_shard[ck][cr:cr + P, :], in_=xl2sb[:])
                        psC3 = pp.tile([P, HC2], F32, tag="psC", bufs=1)
                        for k in range(2):
                            nc.tensor.matmul(psC3[:, 0:HC2],
                                             lhsT=hTt[:, k * P:(k + 1) * P],
                                             rhs=wr2b[k][:],
                                             start=(k == 0), stop=(k == 1))
                        nc.vector.tensor_tensor(
                            out=xr2_loc[:, t * HC2:(t + 1) * HC2],
                            in0=psC3[:, 0:HC2], in1=eb2B[:], op=ALU.add)

                        # chunked AllGather as soon as a chunk's tiles are done
                        for k in range(NCHUNK):
                            if t == CHUNK_T0[k] + CHUNK_TILES[k] - 1:
                                o0 = CHUNK_BASE[k]
                                o1 = o0 + W * CHUNK_TILES[k] * P
                                nc.gpsimd.collective_compute(
                                    "AllGather", ALU.bypass,
                                    ins=[xl2_shard[k][:]],
                                    outs=[xl2_full[o0:o1, :]],
                                    replica_groups=RG)

                # ======== layer 2 edge loop ========
                with (
                    tc.tile_pool(name="l2_w", bufs=4) as wp,
                    tc.tile_pool(name="l2_p", bufs=2, space="PSUM") as pp,
                    tc.tile_pool(name="l2_a", bufs=1, space="PSUM") as pa,
                ):
                    for t in range(NT):
                        b0 = t * bpt
                        relT = wp.tile([P, bpt * P], BF16, tag="relT2")
                        nc.sync.dma_start(
                            out=relT[:],
                            in_=relE[b0 * P:(b0 + bpt) * P, :],
                            transpose=True)

                        acc2t = pa.tile([P, HC2], F32, tag="acc2t", bufs=2)
                        accd2t = pa.tile([P, H], F32, tag="accd2t", bufs=2)
                        acc2 = acc2t[:]
                        accd2 = accd2t[:]
                        XR2 = xr2_loc[:, t * HC2:(t + 1) * HC2]

                        for (j0, kk) in sb_splits(K2):
                            sfx = "" if kk == K2 else f"_{kk}"
                            V2 = wp.tile([P, kk * HC2], BF16, tag=f"V2{sfx}")
                            for jj in range(kk):
                                b = b0 + j0 + jj
                                nc.gpsimd.indirect_dma_start(
                                    out=V2[:, jj * HC2:(jj + 1) * HC2],
                                    out_offset=None, in_=xl2_full[:],
                                    in_offset=IOA(ap=gsrc_t[:, b:b + 1],
                                                  axis=0))
                            Qt = wp.tile([P, kk * P], BF16, tag=f"Qt{sfx}")
                            nc.vector.tensor_tensor(
                                out=Qt[:].rearrange("p (k q) -> p k q", q=P),
                                in0=iotaF[:].rearrange("p (o q) -> p o q", o=1)
                                .to_broadcast([P, kk, P]),
                                in1=gseg_t[:, b0 + j0:b0 + j0 + kk]
                                .rearrange("p (k o) -> p k o", o=1)
                                .to_broadcast([P, kk, P]),
                                op=ALU.is_equal)
                            segB = wp.tile([P, kk * P], BF16, tag=f"segB{sfx}")
                            nc.scalar.dma_start(
                                out=segB[:],
                                in_=gsegT[b0 + j0:b0 + j0 + kk, :]
                                .rearrange("(o k) p -> o (k p)", o=1)
                                .to_broadcast([P, kk * P]))
                            Pbt = wp.tile([P, kk * P], BF16, tag=f"Pbt{sfx}")
                            nc.vector.tensor_scalar(
                                out=Pbt[:], in0=segB[:], scalar1=iotaP[:],
                                scalar2=None, op0=ALU.is_equal)
                            psM = pp.tile([P, kk * HC2], F32, tag=f"psM{sfx}", bufs=1)
                            for jj in range(kk):
                                j = j0 + jj
                                nc.tensor.matmul(
                                    psM[:, jj * HC2:(jj + 1) * HC2],
                                    lhsT=Pbt[:, jj * P:(jj + 1) * P],
                                    rhs=XR2, start=True, stop=False)
                                nc.tensor.matmul(
                                    psM[:, jj * HC2:(jj + 1) * HC2],
                                    lhsT=relT[:, j * P:(j + 1) * P],
                                    rhs=we2b[:], start=False, stop=False)
                                nc.tensor.matmul(
                                    psM[:, jj * HC2:(jj + 1) * HC2],
                                    lhsT=ident[:],
                                    rhs=V2[:, jj * HC2:(jj + 1) * HC2],
                                    start=False, stop=True)
                            Mr = wp.tile([P, kk * HC2], BF16, tag=f"Mr{sfx}")
                            nc.scalar.activation(Mr[:], psM[:], ACTF.Prelu,
                                                 alpha=0.2)
                            Tm = wp.tile([P, kk * HC2], BF16, tag=f"Tm{sfx}")
                            nc.vector.tensor_tensor(
                                out=Tm[:], in0=Mr[:],
                                in1=attB2[:, 0:kk * HC2], op=ALU.mult)
                            logit = wp.tile([P, kk * H], F32, tag=f"lg{sfx}")
                            nc.vector.tensor_reduce(
                                out=logit[:],
                                in_=Tm[:].rearrange("p (q c) -> p q c", c=OUT),
                                axis=mybir.AxisListType.X, op=ALU.add)
                            esc = wp.tile([P, kk * H], BF16, tag=f"esc{sfx}")
                            nc.scalar.activation(esc[:], logit[:], ACTF.Exp)
                            Rv = wp.tile([P, kk * HC2], BF16, tag=f"Rv{sfx}")
                            nc.vector.tensor_tensor(
                                out=Rv[:].rearrange("p (k h c) -> p k h c",
                                                    h=H, c=OUT),
                                in0=V2[:].rearrange("p (k h c) -> p k h c",
                                                    h=H, c=OUT),
                                in1=esc[:].rearrange("p (k h) -> p k h", h=H)
                                .rearrange("p k (h o) -> p k h o", o=1)
                                .to_broadcast([P, kk, H, OUT]),
                                op=ALU.mult)
                            for jj in range(kk):
                                j = j0 + jj
                                nc.tensor.matmul(
                                    acc2, lhsT=Qt[:, jj * P:(jj + 1) * P],
                                    rhs=Rv[:, jj * HC2:(jj + 1) * HC2],
                                    start=(j == 0), stop=(j == bpt - 1))
                                nc.tensor.matmul(
                                    accd2, lhsT=Qt[:, jj * P:(jj + 1) * P],
                                    rhs=esc[:, jj * H:(jj + 1) * H],
                                    start=(j == 0), stop=(j == bpt - 1))

                        # epilogue: out = mean_h(acc_h/denom_h) + bias
                        dn2 = wp.tile([P, H], F32, tag="dn2")
                        nc.vector.tensor_scalar_add(dn2[:], accd2, 1e-20)
                        rec2 = wp.tile([P, H], F32, tag="rec2")
                        nc.vector.reciprocal(rec2[:], dn2[:])
                        rec4 = wp.tile([P, H], F32, tag="rec4")
                        nc.vector.tensor_scalar_mul(rec4[:], rec2[:], 0.25)
                        ho = wp.tile([P, H * OUT], F32, tag="ho")
                        for hh in range(H):
                            nc.scalar.activation(
                                ho[:, hh * OUT:(hh + 1) * OUT],
                                acc2[:, hh * OUT:(hh + 1) * OUT],
                                ACTF.Copy, scale=rec4[:, hh:hh + 1])
                        s01 = wp.tile([P, OUT], F32, tag="s01")
                        nc.vector.tensor_tensor(out=s01[:], in0=ho[:, 0:OUT],
                                                in1=ho[:, OUT:2 * OUT],
                                                op=ALU.add)
                        s23 = wp.tile([P, OUT], F32, tag="s23")
                        nc.gpsimd.tensor_tensor(out=s23[:],
                                                in0=ho[:, 2 * OUT:3 * OUT],
                                                in1=ho[:, 3 * OUT:4 * OUT],
                                                op=ALU.add)
                        s03 = wp.tile([P, OUT], F32, tag="s03")
                        nc.vector.tensor_tensor(out=s03[:], in0=s01[:],
                                                in1=s23[:], op=ALU.add)
                        osb = wp.tile([P, OUT], F32, tag="osb")
                        nc.gpsimd.tensor_tensor(out=osb[:], in0=s03[:],
                                                in1=ob2B[:], op=ALU.add)
                        nc.sync.dma_start(out=out_p[t * P:(t + 1) * P, :],
                                          in_=osb[:])

    nc.compile()
    return nc


def _make_in_maps(inp, pre):
    f32 = np.float32
    lut = pre["lut"]
    perm_pos = pre["perm_pos"]
    x = np.asarray(inp["x"], f32)
    rel_padz = np.zeros((R + 1, IN), BF)
    rel_padz[:R] = np.asarray(inp["relations"], f32).astype(BF)
    a = lambda k: np.asarray(inp[k], f32)
    rep = dict(
        wl1=a("Wl1"), wr1=a("Wr1"), we1=a("We1"),
        att1f=a("att1").reshape(1, HC1),
        eb1=(a("bl1") + a("br1")).reshape(1, HC1),
        ob1=(a("bl1") + a("bias1")).reshape(1, HC1),
        wl2=a("Wl2"), wr2=a("Wr2"), we2=a("We2"),
        att2f=a("att2").reshape(1, HC2),
        eb2=(a("bl2") + a("br2")).reshape(1, HC2),
        ob2=(a("bl2").reshape(H, OUT).mean(axis=0) + a("bias2")).reshape(1, OUT),
    )
    # x in chunk-major slot rows, staged transposed for direct matmul lhsT
    x_slot = np.zeros((NSLOT, IN), BF)      # chunk-major row order
    x_slot[lut[perm_pos]] = x.astype(BF)
    xT_slot = np.ascontiguousarray(x_slot.T)
    in_maps = []
    for c in range(W):
        m = dict(rep)
        m["gsrc"] = np.ascontiguousarray(pre["gsrc"][c])
        m["gseg"] = np.ascontiguousarray(pre["gseg"][c])
        m["gsegT"] = np.ascontiguousarray(pre["gseg"][c].T.astype(BF))
        m["relE"] = np.ascontiguousarray(
            rel_padz[np.minimum(pre["grel"][c].T.reshape(-1), R)])
        m["xT_slot"] = xT_slot
        base = c * NT * P
        m["xT_own"] = np.ascontiguousarray(x_slot[lut[base:base + SHARD]].T)
        in_maps.append(m)
    return in_maps


_CACHE = {}


def kernel(x, edge_index, relations,
           Wl1, bl1, Wr1, br1, We1, att1, bias1,
           Wl2, bl2, Wr2, br2, We2, att2, bias2, **_unused):
    x = np.asarray(x, np.float32)
    edge_index = np.asarray(edge_index)
    relations = np.asarray(relations, np.float32)

    pre = _preprocess(edge_index)
    bpt = pre["bpt"]

    if bpt not in _CACHE:
        _CACHE[bpt] = _build(bpt)
    nc = _CACHE[bpt]

    in_maps = _make_in_maps(
        dict(x=x, relations=relations, Wl1=Wl1, bl1=bl1, Wr1=Wr1, br1=br1,
             We1=We1, att1=att1, bias1=bias1, Wl2=Wl2, bl2=bl2, Wr2=Wr2,
             br2=br2, We2=We2, att2=att2, bias2=bias2), pre)

    import os
    trace = os.environ.get("GAT_TRACE", "0") == "1"
    res = run_bass_kernel_spmd(nc, in_maps, list(range(W)), trace=trace)
    global LAST_EXEC_NS, LAST_RES
    LAST_EXEC_NS = res.exec_time_ns
    LAST_RES = res
    cat = np.concatenate([res.results[c]["out"] for c in range(W)], axis=0)
    return np.ascontiguousarray(cat[pre["perm_pos"]])


if __name__ == "__main__":
    pass

